# revision 1
# baseline (speedup 1.0000x reference)
"""Trainium2 Bass kernel for nn_Estor_concat (scatter_memory).

Math (exact reformulation of the reference):
  v_tag = (tag_emb @ Wv.T + bv) @ out_proj_w.T + out_proj_b            [T, H]
  W_eff[t, j] = sum_h v_tag[t, h] * ff1_w[j, t*H + h]                  [T, H]
  counts[t, b, s] = #spans(tag=t, batch=b) covering s
                  = sum_n onehot_t[n] * ((s < end_n) - (s < start_n))   (PE matmul)
  h1 = counts_b.T @ W_eff + ff1_b ; h2 = relu(h1) @ ff2_w.T + ff2_b
  x = [word_emb_b | h2]; LayerNorm folded into the output projection:
  out = (x @ (lin_w.T * g) - mu * c1) * rstd + (lin_w @ b + lin_b)

Sharding: data-parallel over batch (8 cores, 1 batch each); the W_eff
computation is sharded over tags (2 tags/core) with one AllGather. The
schedule front-loads the W_eff chain so the AllGather (~15us launch
latency) overlaps counts, the word-embedding half of the output/stats
accumulation, and all remaining loads.
"""

import ml_dtypes
import numpy as np

import concourse.bacc as bacc
import concourse.bass as bass
import concourse.mybir as mybir
import concourse.tile as tile
from concourse.bass_utils import run_bass_kernel_spmd

T, B, S, H = 16, 8, 512, 768
H2 = 384
NEW_H = H + H2          # 1152
NL = 33                 # num labels
EPS = 1e-12
NCORES = 8
TPC = T // NCORES       # tags per core = 2
KC_H = H // 128         # 6 chunks of the hidden dim
KC_H2 = H2 // 128       # 3
KC_F = NEW_H // 128     # 9
P = 128
HH = H // 2             # 384 (psum-bank-sized half of H)
ML = 65                 # raw-matmul lhsT cols: [sum | 31 pad | 33 labels]

F32 = mybir.dt.float32
BF16 = mybir.dt.bfloat16
F16 = mybir.dt.float16


def build_kernel(n_span_tiles: int):
    nc = bacc.Bacc(
        "TRN2",
        target_bir_lowering=False,
        debug=False,
        enable_asserts=True,
        num_devices=NCORES,
    )

    def inp(name, shape, dtype=F32):
        return nc.dram_tensor(name, list(shape), dtype, kind="ExternalInput").ap()

    # per-core inputs (host pre-sharded / pre-transposed / pre-chunked)
    we_t = inp("we_t", (P, KC_H, S))            # word_embedding[b].T chunked (f32)
    tag2t = inp("tag2t", (P, KC_H, TPC), BF16)  # tag_emb[2c:2c+2].T chunked
    wv_t = inp("wv_t", (P, KC_H, H), BF16)      # Wv.T chunked [p, hc, h']
    bv_col = inp("bv_col", (P, KC_H))           # bv chunked per-partition
    op_t = inp("op_t", (P, KC_H, H), BF16)      # out_proj_w.T chunked
    ob_col = inp("ob_col", (P, KC_H))
    ff1t_c = inp("ff1t_c", (P, TPC * KC_H, H), BF16)  # ff1_w.T rows (2 tags) chunked
    ff1b_col = inp("ff1b_col", (P, KC_H))
    ff2t = inp("ff2t", (P, KC_H, H2), BF16)     # ff2_w.T chunked
    ff2b_col = inp("ff2b_col", (P, KC_H2))
    g_col = inp("g_col", (P, KC_F))
    lwg2 = inp("lwg2", (P, KC_F, ML), BF16)     # [lin_w.T | 0pad | ones] (g folded on dev)
    lw_b = inp("lw_b", (P, KC_F, NL), BF16)     # lin_w.T (for c2)
    b_col = inp("b_col", (P, KC_F), BF16)
    lin_b = inp("lin_b", (NL, 1))
    sp_start = inp("sp_start", (P, n_span_tiles))
    sp_end = inp("sp_end", (P, n_span_tiles))
    sp_tag = inp("sp_tag", (P, n_span_tiles))
    iota_s = inp("iota_s", (P, S), F16)         # 0..S-1 on every partition
    iota_t = inp("iota_t", (P, T), F16)

    out = nc.dram_tensor("out", [NL, S], F32, kind="ExternalOutput").ap()

    with tile.TileContext(nc) as tc:
        with (
            tc.tile_pool(name="singles", bufs=1) as singles,
            tc.tile_pool(name="spans", bufs=3) as spans,
            tc.tile_pool(name="work", bufs=3) as work,
            tc.tile_pool(name="stats", bufs=1) as stats,
            tc.tile_pool(name="ps_mm", bufs=3, space="PSUM") as ps_mm,
            tc.tile_pool(name="ps_big", bufs=2, space="PSUM") as ps_big,
            tc.tile_pool(name="ps_acc", bufs=1, space="PSUM") as ps_acc,
            tc.tile_pool(name="dram", bufs=1, space="DRAM") as dram,
        ):
            # ---- constants -------------------------------------------------
            ones_col = singles.tile([P, 1], BF16)
            nc.vector.memset(ones_col, 1.0)
            ones_colf = singles.tile([P, 1], F32)
            nc.vector.memset(ones_colf, 1.0)
            eps_t = singles.tile([1, 1], F32)
            nc.vector.memset(eps_t, EPS)
            ones_row = singles.tile([1, NL], F32)
            nc.vector.memset(ones_row, 1.0)
            neg_ones = singles.tile([P, 1], BF16)
            nc.vector.memset(neg_ones, -1.0)
            scratch = singles.tile([1, 1], F32)

            # ---- DMA queue: W_eff-path loads first (they gate the AllGather)
            tag2_sb = singles.tile([P, KC_H, TPC], BF16)
            nc.sync.dma_start(out=tag2_sb, in_=tag2t)
            bv_sb = singles.tile([P, KC_H], F32)
            nc.sync.dma_start(out=bv_sb, in_=bv_col)
            ob_sb = singles.tile([P, KC_H], F32)
            nc.sync.dma_start(out=ob_sb, in_=ob_col)
            # wv/op split across the SP and ACT queues so both land early;
            # ff1 tl0 chunked so the W_eff matmuls track DMA arrivals
            wv_sb = singles.tile([P, KC_H, H], BF16)
            nc.sync.dma_start(out=wv_sb[:, :3, :], in_=wv_t[:, :3, :])
            nc.scalar.dma_start(out=wv_sb[:, 3:, :], in_=wv_t[:, 3:, :])
            op_sb = singles.tile([P, KC_H, H], BF16)
            nc.sync.dma_start(out=op_sb[:, :3, :], in_=op_t[:, :3, :])
            nc.scalar.dma_start(out=op_sb[:, 3:, :], in_=op_t[:, 3:, :])
            ff1_sb = singles.tile([P, TPC * KC_H, H], BF16)
            for kk in range(KC_H):
                nc.sync.dma_start(
                    out=ff1_sb[:, kk, :], in_=ff1t_c[:, kk, :]
                )
            nc.scalar.dma_start(
                out=ff1_sb[:, KC_H:2 * KC_H, :], in_=ff1t_c[:, KC_H:2 * KC_H, :]
            )


            iota_s_sb = singles.tile([P, S], F16)
            nc.gpsimd.dma_start(out=iota_s_sb, in_=iota_s)
            iota_t_sb = singles.tile([P, T], F16)
            nc.gpsimd.dma_start(out=iota_t_sb, in_=iota_t)
            sps_sb = singles.tile([P, n_span_tiles], F32)
            spe_sb = singles.tile([P, n_span_tiles], F32)
            spt_sb = singles.tile([P, n_span_tiles], F32)
            nc.gpsimd.dma_start(out=sps_sb, in_=sp_start)
            nc.gpsimd.dma_start(out=spe_sb, in_=sp_end)
            nc.gpsimd.dma_start(out=spt_sb, in_=sp_tag)

            ff1b_sb = singles.tile([P, KC_H], F32)
            nc.sync.dma_start(out=ff1b_sb, in_=ff1b_col)
            ff2b_sb = singles.tile([P, KC_H2], F32)
            nc.sync.dma_start(out=ff2b_sb, in_=ff2b_col)
            lwg2_in = singles.tile([P, KC_F, ML], BF16)
            nc.sync.dma_start(out=lwg2_in, in_=lwg2)
            lw_sb = singles.tile([P, KC_F, NL], BF16)
            nc.sync.dma_start(out=lw_sb, in_=lw_b)
            g_sb = singles.tile([P, KC_F], F32)
            nc.sync.dma_start(out=g_sb, in_=g_col)
            b_sb = singles.tile([P, KC_F], BF16)
            nc.sync.dma_start(out=b_sb, in_=b_col)
            linb_sb = singles.tile([NL, 1], F32)
            nc.sync.dma_start(out=linb_sb, in_=lin_b)
            we_sb = singles.tile([P, KC_H, S], F32)
            nc.sync.dma_start(out=we_sb, in_=we_t)
            ff2_sb = singles.tile([P, KC_H, H2], BF16)
            nc.sync.dma_start(out=ff2_sb, in_=ff2t)

            # ================= overlapped with the AllGather =================
            # ---- counts: masks on DVE, accumulate on PE --------------------
            counts_ps = ps_acc.tile([T, S], F32, tag="counts")
            for i in range(n_span_tiles):
                # coverage mask = (s < end) - (s < start); the subtraction is
                # folded into the PE accumulation via a negated onehot.
                lt_e = spans.tile([P, S], BF16, tag="lt_e")
                lt_s = spans.tile([P, S], BF16, tag="lt_s")
                mask = spans.tile([P, S], BF16, tag="mask")
                nc.vector.tensor_scalar(
                    out=lt_e, in0=iota_s_sb, scalar1=spe_sb[:, i:i + 1], scalar2=None,
                    op0=mybir.AluOpType.is_lt,
                )
                nc.vector.tensor_scalar(
                    out=lt_s, in0=iota_s_sb, scalar1=sps_sb[:, i:i + 1], scalar2=None,
                    op0=mybir.AluOpType.is_ge,
                )
                nc.vector.tensor_mul(out=mask, in0=lt_e, in1=lt_s)
                onehot = spans.tile([P, T], BF16, tag="onehot")
                nc.vector.tensor_scalar(
                    out=onehot, in0=iota_t_sb, scalar1=spt_sb[:, i:i + 1], scalar2=None,
                    op0=mybir.AluOpType.is_equal,
                )
                nc.tensor.matmul(
                    counts_ps, onehot, mask,
                    start=(i == 0), stop=(i == n_span_tiles - 1),
                )
            # ---- W_eff chain ----------------------------------------------
            def mmT_2xH(w_sb, rhs_chunks, bias_col, dst_sb, pfx):
                """dst[p, jc, t] = sum_h w[h, j] * rhs[h, t] + bias[j]: result
                arrives already transposed (j on partitions)."""
                for jc in range(KC_H):
                    ps = ps_mm.tile([P, TPC], F32, tag="mm", name=f"{pfx}{jc}")
                    for hc in range(KC_H):
                        nc.tensor.matmul(
                            ps,
                            w_sb[:, hc, jc * P:(jc + 1) * P],
                            rhs_chunks[hc],
                            start=(hc == 0),
                            stop=(hc == KC_H - 1),
                        )
                    nc.vector.tensor_scalar(
                        out=dst_sb[:, jc, :], in0=ps,
                        scalar1=bias_col[:, jc:jc + 1], scalar2=None,
                        op0=mybir.AluOpType.add,
                    )

            vT_sb = singles.tile([P, KC_H, TPC], BF16)
            mmT_2xH(wv_sb, [tag2_sb[:, hc, :] for hc in range(KC_H)], bv_sb,
                    vT_sb, "psv")
            vtT_sb = singles.tile([P, KC_H, TPC], BF16)
            mmT_2xH(op_sb, [vT_sb[:, hc, :] for hc in range(KC_H)], ob_sb,
                    vtT_sb, "psvt")

            # W_eff local rows: W[tl, j] = sum_h vt[tl, h] * ff1T[tl*H + h, j]
            wloc_sb = singles.tile([1, TPC * H], BF16)
            for tl in range(TPC):
                pss = [ps_mm.tile([1, HH], F32, tag="mm", name=f"ps_w{tl}_{nn}")
                       for nn in range(2)]
                for kk in range(KC_H):
                    for nn in range(2):
                        nc.tensor.matmul(
                            pss[nn],
                            vtT_sb[:, kk, tl:tl + 1],
                            ff1_sb[:, tl * KC_H + kk, nn * HH:(nn + 1) * HH],
                            start=(kk == 0),
                            stop=(kk == KC_H - 1),
                        )
                for nn in range(2):
                    nc.vector.tensor_copy(
                        out=wloc_sb[:, tl * H + nn * HH:tl * H + (nn + 1) * HH],
                        in_=pss[nn],
                    )

            # AllGather W_eff: [TPC, H] per core -> [T, H].  Bounce DMAs ride
            # the gpsimd queue (SP's FIFO is full of bulk loads).
            ag_in = dram.tile([1, TPC * H], BF16)
            ag_out = dram.tile([T, H], BF16)
            nc.gpsimd.dma_start(out=ag_in, in_=wloc_sb)
            nc.gpsimd.collective_compute(
                "AllGather",
                mybir.AluOpType.bypass,
                replica_groups=[list(range(NCORES))],
                ins=[ag_in.opt()],
                outs=[ag_out.opt()],
            )
            weff_sb = singles.tile([T, H], BF16)
            nc.sync.dma_start(out=weff_sb[:, :HH], in_=ag_out[:, :HH])
            nc.sync.dma_start(out=weff_sb[:, HH:], in_=ag_out[:, HH:])

            counts_sb = singles.tile([T, S], BF16)
            nc.vector.tensor_copy(out=counts_sb, in_=counts_ps)

            # ---- lwg prep + c1/c2 ------------------------------------------
            lwg2_sb = singles.tile([P, KC_F, ML], BF16)
            lwg2f_sb = singles.tile([P, KC_H, ML], F32)
            for fc in range(KC_F):
                nc.vector.tensor_copy(
                    out=lwg2_sb[:, fc, NL:], in_=lwg2_in[:, fc, NL:]
                )
                nc.vector.tensor_scalar_mul(
                    out=lwg2_sb[:, fc, 0:NL], in0=lwg2_in[:, fc, 0:NL],
                    scalar1=g_sb[:, fc:fc + 1],
                )
            for fc in range(KC_H):
                nc.vector.tensor_copy(
                    out=lwg2f_sb[:, fc, NL:], in_=lwg2_in[:, fc, NL:]
                )
                nc.vector.tensor_scalar_mul(
                    out=lwg2f_sb[:, fc, 0:NL], in0=lwg2_in[:, fc, 0:NL],
                    scalar1=g_sb[:, fc:fc + 1],
                )
            psc1 = ps_mm.tile([1, NL], F32, tag="mm")
            psc2 = ps_mm.tile([NL, 1], F32, tag="mm")
            for fc in range(KC_F):
                nc.tensor.matmul(
                    psc1, neg_ones, lwg2_sb[:, fc, 0:NL],
                    start=(fc == 0), stop=(fc == KC_F - 1),
                )
                nc.tensor.matmul(
                    psc2, lw_sb[:, fc, :], b_sb[:, fc:fc + 1],
                    start=(fc == 0), stop=(fc == KC_F - 1),
                )
            c1n_sb = singles.tile([1, NL], F32)
            nc.vector.tensor_copy(out=c1n_sb, in_=psc1)
            c2_sb = singles.tile([NL, 1], F32)
            nc.vector.tensor_add(out=c2_sb, in0=psc2, in1=linb_sb)

            # ---- word-embedding part of raw / sum / sumsq (fc = 0..5) ------
            pr_we = ps_acc.tile([ML, S], F32, tag="pr")
            ss_we = ps_acc.tile([1, S], F32, tag="ss")
            for fc in range(KC_H):
                nc.tensor.matmul(
                    pr_we, lwg2f_sb[:, fc, :], we_sb[:, fc, :],
                    start=(fc == 0), stop=(fc == KC_H - 1),
                )
                sq = work.tile([P, S], BF16, tag="sq")
                nc.scalar.square(out=sq, in_=we_sb[:, fc, :])
                nc.tensor.matmul(
                    ss_we, ones_col, sq,
                    start=(fc == 0), stop=(fc == KC_H - 1),
                )
            # park the word-embedding halves in SBUF (frees their psum banks
            # and keeps every accumulation group contiguous and same-dtype)
            prwe_sb = singles.tile([ML, S], F32)
            nc.vector.tensor_copy(out=prwe_sb, in_=pr_we)
            sswe_sb = singles.tile([1, S], F32)
            nc.vector.tensor_copy(out=sswe_sb, in_=ss_we)
            # prefetch the Relu table while the collective is in flight
            nc.scalar.activation(
                out=scratch, in_=eps_t,
                func=mybir.ActivationFunctionType.Relu,
            )

            # ================= post-AllGather tail ==========================
            # h1 = relu(counts.T @ W_eff + ff1_b), stored transposed [H, S]
            h1r_sb = singles.tile([P, KC_H, S], BF16)
            for kj in range(KC_H):
                ps = ps_big.tile([P, S], F32, tag="big")
                nc.tensor.matmul(
                    ps, weff_sb[:, kj * P:(kj + 1) * P], counts_sb,
                    start=True, stop=True,
                )
                if kj % 2 == 0:
                    nc.scalar.activation(
                        out=h1r_sb[:, kj, :], in_=ps,
                        func=mybir.ActivationFunctionType.Relu,
                        bias=ff1b_sb[:, kj:kj + 1], scale=1.0,
                    )
                else:
                    nc.vector.tensor_scalar(
                        out=h1r_sb[:, kj, :], in0=ps,
                        scalar1=ff1b_sb[:, kj:kj + 1], scalar2=0.0,
                        op0=mybir.AluOpType.add, op1=mybir.AluOpType.max,
                    )
            # prefetch the Sqrt table before the stats need it
            nc.scalar.activation(
                out=scratch, in_=eps_t,
                func=mybir.ActivationFunctionType.Sqrt,
            )

            # h2 = relu_h1 @ ff2.T + ff2_b, stored transposed [H2, S]
            xh2_sb = singles.tile([P, KC_H2, S], BF16)
            for mc in range(KC_H2):
                ps = ps_big.tile([P, S], F32, tag="big")
                for kj in range(KC_H):
                    nc.tensor.matmul(
                        ps,
                        ff2_sb[:, kj, mc * P:(mc + 1) * P],
                        h1r_sb[:, kj, :],
                        start=(kj == 0), stop=(kj == KC_H - 1),
                    )
                nc.vector.tensor_scalar(
                    out=xh2_sb[:, mc, :], in0=ps,
                    scalar1=ff2b_sb[:, mc:mc + 1], scalar2=None,
                    op0=mybir.AluOpType.add,
                )

            # ---- h2 part of raw / sum / sumsq (fc = 6..8) ------------------
            pr_h2 = ps_acc.tile([ML, S], F32, tag="counts")
            ss_h2 = ps_acc.tile([1, S], F32, tag="ss")
            for mc in range(KC_H2):
                fc = KC_H + mc
                nc.tensor.matmul(
                    pr_h2, lwg2_sb[:, fc, :], xh2_sb[:, mc, :],
                    start=(mc == 0), stop=(mc == KC_H2 - 1),
                )
                sq = work.tile([P, S], BF16, tag="sq")
                nc.vector.tensor_mul(
                    out=sq, in0=xh2_sb[:, mc, :], in1=xh2_sb[:, mc, :]
                )
                nc.tensor.matmul(
                    ss_h2, ones_col, sq,
                    start=(mc == 0), stop=(mc == KC_H2 - 1),
                )

            # ---- stats ------------------------------------------------------
            sum_sb = stats.tile([1, S], F32, tag="sum")
            nc.vector.tensor_add(
                out=sum_sb, in0=pr_h2[ML - 1:ML, :], in1=prwe_sb[ML - 1:ML, :]
            )
            mu_sb = stats.tile([1, S], F32, tag="mu")
            nc.vector.tensor_scalar_mul(out=mu_sb, in0=sum_sb, scalar1=1.0 / NEW_H)
            sst_sb = stats.tile([1, S], F32, tag="sst")
            nc.vector.tensor_add(out=sst_sb, in0=ss_h2, in1=sswe_sb)
            ex2_sb = stats.tile([1, S], F32, tag="ex2")
            nc.vector.tensor_scalar_mul(out=ex2_sb, in0=sst_sb, scalar1=1.0 / NEW_H)
            # raw = we part + h2 part
            a_sb = stats.tile([NL, S], F32, tag="araw")
            nc.vector.tensor_add(
                out=a_sb, in0=pr_h2[0:NL, :], in1=prwe_sb[0:NL, :]
            )
            # -c1 (x) mu as its own (clean) K=1 accumulation
            c1mu_ps = ps_big.tile([NL, S], F32, tag="big")
            nc.tensor.matmul(c1mu_ps, c1n_sb, mu_sb, start=True, stop=True)
            x1_sb = stats.tile([NL, S], F32, tag="x1")
            nc.vector.tensor_add(out=x1_sb, in0=c1mu_ps, in1=a_sb)

            mu2_sb = stats.tile([1, S], F32, tag="mu2")
            nc.vector.tensor_mul(out=mu2_sb, in0=mu_sb, in1=mu_sb)
            var_sb = stats.tile([1, S], F32, tag="var")
            nc.vector.tensor_sub(out=var_sb, in0=ex2_sb, in1=mu2_sb)
            sd_sb = stats.tile([1, S], F32, tag="sd")
            nc.scalar.activation(
                out=sd_sb, in_=var_sb, func=mybir.ActivationFunctionType.Sqrt,
                bias=eps_t, scale=1.0,
            )
            rstd_sb = stats.tile([1, S], F32, tag="rstd")
            nc.vector.reciprocal(out=rstd_sb, in_=sd_sb)

            # broadcast rstd across NL partitions via a K=1 matmul
            rb_ps = ps_big.tile([NL, S], F32, tag="big")
            nc.tensor.matmul(rb_ps, ones_row, rstd_sb, start=True, stop=True)

            # final = (raw - c1*mu) * rstd + c2
            t2_sb = stats.tile([NL, S], F32, tag="t2")
            nc.vector.tensor_mul(out=t2_sb, in0=rb_ps, in1=x1_sb)
            f_sb = stats.tile([NL, S], F32, tag="fin")
            nc.vector.tensor_scalar(
                out=f_sb, in0=t2_sb, scalar1=c2_sb, scalar2=None,
                op0=mybir.AluOpType.add,
            )
            nc.sync.dma_start(out=out, in_=f_sb)

    nc.compile()
    return nc


def _chunked(a, kc):
    """[kc*128, N...] -> [128, kc, N...] (partition-major chunk layout)."""
    return np.ascontiguousarray(
        a.reshape(kc, P, *a.shape[1:]).transpose(1, 0, *range(2, a.ndim + 1))
    )


_CACHE = {}


def kernel(**inputs) -> np.ndarray:
    bfl = ml_dtypes.bfloat16
    we = np.asarray(inputs["word_embedding"], np.float32)
    te = np.asarray(inputs["tag_embedding"], np.float32)
    ipw = np.asarray(inputs["in_proj_w"], np.float32)
    ipb = np.asarray(inputs["in_proj_b"], np.float32)
    opw = np.asarray(inputs["out_proj_w"], np.float32)
    ob_ = np.asarray(inputs["out_proj_b"], np.float32)
    f1w = np.asarray(inputs["ff1_w"], np.float32)
    f1b = np.asarray(inputs["ff1_b"], np.float32)
    f2w = np.asarray(inputs["ff2_w"], np.float32)
    f2b = np.asarray(inputs["ff2_b"], np.float32)
    lg = np.asarray(inputs["ln_g"], np.float32)
    lb = np.asarray(inputs["ln_b"], np.float32)
    lw = np.asarray(inputs["lin_w"], np.float32)
    lbias = np.asarray(inputs["lin_b"], np.float32)
    sb = np.asarray(inputs["span_batch"]).astype(np.int64)
    st = np.asarray(inputs["span_tag"]).astype(np.int64)
    ss = np.asarray(inputs["span_start"]).astype(np.int64)
    se = np.asarray(inputs["span_end"]).astype(np.int64)

    # ---- host-side sharding / layout prep -----------------------------
    counts_per_b = np.bincount(sb, minlength=B)
    n_span_tiles = max(1, int(np.ceil(counts_per_b.max() / P)))
    n_pad = n_span_tiles * P

    wv_t = _chunked(ipw[2 * H:].T.astype(bfl), KC_H)        # [P, KC_H, H]
    bv_col = np.ascontiguousarray(ipb[2 * H:].reshape(KC_H, P).T)
    op_t = _chunked(opw.T.astype(bfl), KC_H)
    ob_col = np.ascontiguousarray(ob_.reshape(KC_H, P).T)
    ff1T = f1w.T.astype(bfl)                                # [T*H, H]
    ff2t = _chunked(f2w.T.astype(bfl), KC_H)                # [P, KC_H, H2]
    ff1b_col = np.ascontiguousarray(f1b.reshape(KC_H, P).T)
    ff2b_col = np.ascontiguousarray(f2b.reshape(KC_H2, P).T)
    g_col = np.ascontiguousarray(lg.reshape(KC_F, P).T)
    b_col = np.ascontiguousarray(lb.reshape(KC_F, P).T.astype(bfl))
    lwt = lw.T.astype(bfl)                                  # [NEW_H, NL]
    lw_b = _chunked(lwt, KC_F)                              # [P, KC_F, NL]
    lwg2 = np.zeros((P, KC_F, ML), bfl)
    lwg2[:, :, ML - 1] = 1.0
    lwg2[:, :, 0:NL] = lw_b
    lin_b_col = np.ascontiguousarray(lbias.reshape(NL, 1))
    iota_s = np.ascontiguousarray(
        np.broadcast_to(np.arange(S, dtype=np.float16), (P, S))
    )
    iota_t = np.ascontiguousarray(
        np.broadcast_to(np.arange(T, dtype=np.float16), (P, T))
    )

    in_maps = []
    for c in range(NCORES):
        idx = np.where(sb == c)[0]
        n = len(idx)
        sps = np.zeros(n_pad, np.float32)
        spe = np.zeros(n_pad, np.float32)
        spt = np.zeros(n_pad, np.float32)
        sps[:n] = ss[idx]
        spe[:n] = se[idx]
        spt[:n] = st[idx]
        in_maps.append(dict(
            we_t=_chunked(np.ascontiguousarray(we[c].T), KC_H),
            tag2t=_chunked(te[c * TPC:(c + 1) * TPC].T.astype(bfl), KC_H),
            wv_t=wv_t, bv_col=bv_col, op_t=op_t, ob_col=ob_col,
            ff1t_c=_chunked(
                ff1T[c * TPC * H:(c + 1) * TPC * H], TPC * KC_H
            ),
            ff1b_col=ff1b_col, ff2t=ff2t, ff2b_col=ff2b_col,
            g_col=g_col, lwg2=lwg2, lw_b=lw_b, b_col=b_col, lin_b=lin_b_col,
            sp_start=np.ascontiguousarray(sps.reshape(n_span_tiles, P).T),
            sp_end=np.ascontiguousarray(spe.reshape(n_span_tiles, P).T),
            sp_tag=np.ascontiguousarray(spt.reshape(n_span_tiles, P).T),
            iota_s=iota_s, iota_t=iota_t,
        ))

    if n_span_tiles not in _CACHE:
        _CACHE[n_span_tiles] = build_kernel(n_span_tiles)
    nc = _CACHE[n_span_tiles]

    res = run_bass_kernel_spmd(nc, in_maps, list(range(NCORES)))
    out = np.stack([res.results[c]["out"].T for c in range(NCORES)])
    return out.astype(np.float32)


if __name__ == "__main__":
    import reference
    inp = {k: np.asarray(v) for k, v in reference.setup_inputs().items()}
    got = kernel(**inp)
    print("kernel output:", got.shape, got.dtype)



# revision 9
# speedup vs baseline: 1.6703x; 1.6703x over previous
"""Trainium2 Bass kernel for nn_Estor_concat (scatter_memory).

Fully-local formulation (no collective, no cross-core traffic):
  v      = tag_emb @ Wv.T + bv                       [T, H]   (bf16)
  v_tag  = (v @ out_proj_w.T + out_proj_b) / 256     [T, H]   (scale folded
           into the out_proj weights host-side)
  W_eff[t, j] = sum_h v_tag[t, h] * ff1qT[t*H+h, j]  [T, H]
           where ff1qT = ff1_w.T * 256 quantized to fp8-e4m3; every core
           computes the FULL W_eff from the fp8 matrix (9.4 MB/core)
           instead of AllGather-ing tag shards (the collective's fixed
           ~15 us launch cost dominates any sharded variant).
  counts[t, s] = #spans covering s = PE-accumulated (onehot x (iota<end))
           minus (onehot x (iota<start)) over 128-span tiles.
  h1 = relu(W_eff.T @ counts + b1); h2 = ff2 @ h1 + b2
  LayerNorm + output projection evaluated TRANSPOSED (positions on
  partitions) so the stats chain is partition-parallel:
    rawT[s, l] = sum_f x[f, s]*lwg[f, l] + mu[s]*c1[l]   (rank-1 mu update
           via a K=1 matmul; lwg = lin_w.T * ln_g, c1 = -sum_f lwg)
    out[s, l] = rawT[s, l] * rsqrt(var[s] + eps) + c2[l]

Sharding: pure data-parallel over batch (core c owns batch c); all
weights replicated. DMA is spread over the three parallel queues
(SP / Activation / Pool), with the fp8 ff1 sliced per j-chunk so the
W_eff -> h1 -> h2 pipeline consumes slices as they land.
"""

import ml_dtypes
import numpy as np

import concourse.bacc as bacc
import concourse.bass as bass
import concourse.mybir as mybir
import concourse.tile as tile
from concourse.bass_utils import run_bass_kernel_spmd

T, B, S, H = 16, 8, 512, 768
H2 = 384
NEW_H = H + H2          # 1152
NL = 33                 # num labels
EPS = 1e-12
NCORES = 8
KC_H = H // 128         # 6
KC_H2 = H2 // 128       # 3
KC_F = NEW_H // 128     # 9
NCS = S // 128          # 4 position chunks
P = 128
FF1_SCALE = 256.0

F32 = mybir.dt.float32
BF16 = mybir.dt.bfloat16
F16 = mybir.dt.float16
FP8 = mybir.dt.float8e4

RELU = mybir.ActivationFunctionType.Relu
SQRT = mybir.ActivationFunctionType.Sqrt
SQUARE = mybir.ActivationFunctionType.Square


def build_kernel(n_span_tiles: int):
    nc = bacc.Bacc(
        "TRN2",
        target_bir_lowering=False,
        debug=False,
        enable_asserts=True,
        num_devices=NCORES,
    )

    def inp(name, shape, dtype=F32):
        return nc.dram_tensor(name, list(shape), dtype, kind="ExternalInput").ap()

    wv_t = inp("wv_t", (P, KC_H, H), BF16)       # Wv.T chunked [h, hc, h']
    bv_col = inp("bv_col", (P, KC_H))
    ops_t = inp("ops_t", (P, KC_H, H), BF16)     # out_proj.T / 256 chunked
    obs_col = inp("obs_col", (P, KC_H))          # out_proj_b / 256
    tagT = inp("tagT", (P, KC_H, T), BF16)       # tag_emb.T chunked
    ff1q = inp("ff1q", (P, KC_H, T * KC_H, P), FP8)  # ff1.T*256 [h, jc, t*6+hc, j]
    ff1b_col = inp("ff1b_col", (P, KC_H))
    ff2t = inp("ff2t", (P, KC_H, H2), BF16)      # ff2.T chunked
    ff2b_col = inp("ff2b_col", (P, KC_H2))
    we_t = inp("we_t", (P, KC_H, S), BF16)       # word_embedding[b].T chunked
    lwg = inp("lwg", (P, KC_F, NL), BF16)        # (lin_w * ln_g).T chunked
    c1b = inp("c1b", (P, NL))                    # -sum_f lwg[f, :] (bcast)
    c2b = inp("c2b", (P, NL))                    # lin_w @ ln_b + lin_b (bcast)
    ident = inp("ident", (P, P), BF16)           # transpose identity
    sp_start = inp("sp_start", (P, n_span_tiles))
    sp_end = inp("sp_end", (P, n_span_tiles))
    sp_tag = inp("sp_tag", (P, n_span_tiles))
    iota_s = inp("iota_s", (P, S), F16)
    iota_t = inp("iota_t", (P, T), F16)

    out = nc.dram_tensor("out", [P, NCS, NL], F32, kind="ExternalOutput").ap()

    with tile.TileContext(nc) as tc:
        with (
            tc.tile_pool(name="singles", bufs=1) as singles,
            tc.tile_pool(name="spans", bufs=3) as spans,
            tc.tile_pool(name="ps_h2", bufs=1, space="PSUM") as ps_h2,
            tc.tile_pool(name="ps_big", bufs=1, space="PSUM") as ps_big,
            tc.tile_pool(name="ps_acc", bufs=1, space="PSUM") as ps_acc,
            tc.tile_pool(name="ps_sm", bufs=1, space="PSUM") as ps_sm,
        ):
            # ---- tiny constants -------------------------------------------
            ones_col = singles.tile([P, 1], BF16)
            nc.vector.memset(ones_col, 1.0)
            eps_t = singles.tile([1, 1], F32)
            nc.vector.memset(eps_t, EPS)
            eps_col = singles.tile([P, 1], F32)
            nc.vector.memset(eps_col, EPS)
            scratch = singles.tile([1, 1], F32)

            # ---- DMA schedule ---------------------------------------------
            # Three parallel queues. Priority: wv/op (gate the v-chain),
            # then ff1 jc-slices in consumption order (each split 3 ways),
            # with we/ff2/lwg slotted between.
            wv_sb = singles.tile([P, KC_H, H], BF16)
            ops_sb = singles.tile([P, KC_H, H], BF16)
            tag_sb = singles.tile([P, KC_H, T], BF16)
            bv_sb = singles.tile([P, KC_H], F32)
            obs_sb = singles.tile([P, KC_H], F32)
            ff1b_sb = singles.tile([P, KC_H], F32)
            ff2b_sb = singles.tile([P, KC_H2], F32)
            iota_s_sb = singles.tile([P, S], F16)
            iota_t_sb = singles.tile([P, T], F16)
            sps_sb = singles.tile([P, n_span_tiles], F32)
            spe_sb = singles.tile([P, n_span_tiles], F32)
            spt_sb = singles.tile([P, n_span_tiles], F32)
            ident_sb = singles.tile([P, P], BF16)
            lwg_sb = singles.tile([P, KC_F, NL], BF16)
            c1b_sb = singles.tile([P, NL], F32)
            c2b_sb = singles.tile([P, NL], F32)
            we_sb = singles.tile([P, KC_H, S], BF16)
            ff2_sb = singles.tile([P, KC_H, H2], BF16)
            ff1_sb = singles.tile([P, KC_H, T * KC_H, P], FP8)

            # Pool: small tensors first, then ff1 thirds
            nc.gpsimd.dma_start(out=iota_s_sb, in_=iota_s)
            nc.gpsimd.dma_start(out=iota_t_sb, in_=iota_t)
            nc.gpsimd.dma_start(out=sps_sb, in_=sp_start)
            nc.gpsimd.dma_start(out=spe_sb, in_=sp_end)
            nc.gpsimd.dma_start(out=spt_sb, in_=sp_tag)
            nc.gpsimd.dma_start(out=tag_sb, in_=tagT)
            nc.gpsimd.dma_start(out=bv_sb, in_=bv_col)
            nc.gpsimd.dma_start(out=obs_sb, in_=obs_col)
            nc.gpsimd.dma_start(out=ff1b_sb, in_=ff1b_col)
            nc.gpsimd.dma_start(out=ff2b_sb, in_=ff2b_col)
            nc.gpsimd.dma_start(out=ident_sb, in_=ident)
            nc.gpsimd.dma_start(out=c1b_sb, in_=c1b)
            nc.gpsimd.dma_start(out=c2b_sb, in_=c2b)
            nc.gpsimd.dma_start(out=lwg_sb, in_=lwg)

            # SP: wv, we, then ff1 thirds
            nc.sync.dma_start(out=wv_sb, in_=wv_t)
            nc.sync.dma_start(out=we_sb, in_=we_t)
            # Act: op, ff2, then ff1 thirds
            nc.scalar.dma_start(out=ops_sb, in_=ops_t)
            nc.scalar.dma_start(out=ff2_sb, in_=ff2t)

            # ff1 jc-slices, each split into thirds along the chunk dim (96)
            G = T * KC_H  # 96
            g3 = G // 3
            for jc in range(KC_H):
                nc.gpsimd.dma_start(
                    out=ff1_sb[:, jc, 0:g3, :], in_=ff1q[:, jc, 0:g3, :]
                )
                nc.sync.dma_start(
                    out=ff1_sb[:, jc, g3:2 * g3, :], in_=ff1q[:, jc, g3:2 * g3, :]
                )
                nc.scalar.dma_start(
                    out=ff1_sb[:, jc, 2 * g3:G, :], in_=ff1q[:, jc, 2 * g3:G, :]
                )

            # prefetch the activation table (Relu/Square/Rsqrt in one set)
            nc.scalar.activation(out=scratch, in_=eps_t, func=SQRT)

            # ---- counts on (DVE compares + PE accumulation) ---------------
            # counts[t, s] = sum_spans onehot_t * [(iota < end) - (iota < start)]
            counts_ps = ps_acc.tile([T, S], F32, tag="counts")
            n_mm = 0
            for i in range(n_span_tiles):
                lt_e = spans.tile([P, S], BF16, tag="lt_e")
                lt_s = spans.tile([P, S], BF16, tag="lt_s")
                nc.vector.tensor_scalar(
                    out=lt_e, in0=iota_s_sb, scalar1=spe_sb[:, i:i + 1],
                    scalar2=None, op0=mybir.AluOpType.is_lt,
                )
                nc.vector.tensor_scalar(
                    out=lt_s, in0=iota_s_sb, scalar1=sps_sb[:, i:i + 1],
                    scalar2=None, op0=mybir.AluOpType.is_lt,
                )
                oh_p = spans.tile([P, T], BF16, tag="oh_p")
                oh_n = spans.tile([P, T], BF16, tag="oh_n")
                nc.vector.tensor_scalar(
                    out=oh_p, in0=iota_t_sb, scalar1=spt_sb[:, i:i + 1],
                    scalar2=None, op0=mybir.AluOpType.is_equal,
                )
                nc.vector.tensor_scalar(
                    out=oh_n, in0=iota_t_sb, scalar1=spt_sb[:, i:i + 1],
                    scalar2=-1.0, op0=mybir.AluOpType.is_equal,
                    op1=mybir.AluOpType.mult,
                )
                nc.tensor.matmul(
                    counts_ps, oh_p, lt_e,
                    start=(n_mm == 0), stop=False,
                )
                n_mm += 1
                nc.tensor.matmul(
                    counts_ps, oh_n, lt_s,
                    start=False, stop=(i == n_span_tiles - 1),
                )
                n_mm += 1
            counts_sb = singles.tile([T, S], BF16)
            nc.vector.tensor_copy(out=counts_sb, in_=counts_ps)

            # ---- v chain: vT then vtT (scaled), both [h-part, hc, t] ------
            vT_sb = singles.tile([P, KC_H, T], BF16)
            for jc in range(KC_H):
                ps = ps_sm.tile([P, T], F32, tag="sm", name=f"psv{jc}")
                for hc in range(KC_H):
                    nc.tensor.matmul(
                        ps, wv_sb[:, hc, jc * P:(jc + 1) * P], tag_sb[:, hc, :],
                        start=(hc == 0), stop=(hc == KC_H - 1),
                    )
                nc.vector.tensor_scalar(
                    out=vT_sb[:, jc, :], in0=ps,
                    scalar1=bv_sb[:, jc:jc + 1], scalar2=None,
                    op0=mybir.AluOpType.add,
                )
            vtT_sb = singles.tile([P, KC_H, T], BF16)
            for jc in range(KC_H):
                ps = ps_sm.tile([P, T], F32, tag="sm", name=f"psvt{jc}")
                for hc in range(KC_H):
                    nc.tensor.matmul(
                        ps, ops_sb[:, hc, jc * P:(jc + 1) * P], vT_sb[:, hc, :],
                        start=(hc == 0), stop=(hc == KC_H - 1),
                    )
                nc.vector.tensor_scalar(
                    out=vtT_sb[:, jc, :], in0=ps,
                    scalar1=obs_sb[:, jc:jc + 1], scalar2=None,
                    op0=mybir.AluOpType.add,
                )

            # ---- persistent accumulators ----------------------------------
            h2_ps = ps_h2.tile([P, KC_H2, S], F32)          # 3 banks
            # one bank: [cs, 0:NL] = rawT, [cs, NL:NL+2] = (sum, sumsq).
            # The whole bank is ONE accumulation group (psum zero regions are
            # bank-granular): an explicit zeroing matmul opens it, every
            # rawT/sums matmul joins with start=False, and the final mu
            # rank-1 update closes it.
            acc_ps = ps_acc.tile([P, NCS, NL + 2], F32)
            rawT_ps = [acc_ps[:, cs, 0:NL] for cs in range(NCS)]
            sums_ps = [acc_ps[:, cs, NL:NL + 2] for cs in range(NCS)]
            zrow = singles.tile([1, NCS * (NL + 2)], BF16)
            nc.vector.memset(zrow, 0.0)
            nc.tensor.matmul(
                acc_ps[:, :, :], zrow[:, 0:P], zrow,
                start=True, stop=False,
            )

            # we-part squares (DVE) — needed only for the sums
            sqwe_sb = singles.tile([P, KC_H, S], BF16)

            # ---- per-jc pipeline: W_eff -> transpose -> h1 -> h2 accum ----
            h1r_sb = singles.tile([P, KC_H, S], BF16)
            for jc in range(KC_H):
                wps = ps_sm.tile([P, T], F32, tag="sm", name=f"wps{jc}")
                for t in range(T):
                    for hc in range(KC_H):
                        nc.tensor.matmul(
                            wps[:, t:t + 1],
                            ff1_sb[:, jc, t * KC_H + hc, :],
                            vtT_sb[:, hc, t:t + 1],
                            start=(hc == 0), stop=(hc == KC_H - 1),
                        )
                wbf = spans.tile([P, T], BF16, tag="wbf")
                nc.vector.tensor_copy(out=wbf, in_=wps)
                tp = ps_sm.tile([T, P], BF16, tag="tp", name=f"tp{jc}")
                nc.tensor.transpose(tp, wbf, ident_sb)
                wrow = spans.tile([T, P], BF16, tag="wrow")
                nc.vector.tensor_copy(out=wrow, in_=tp)
                h1p = ps_big.tile([P, S], F32, tag="big", name=f"h1p{jc}")
                nc.tensor.matmul(h1p, wrow, counts_sb, start=True, stop=True)
                nc.vector.tensor_scalar(
                    out=h1r_sb[:, jc, :], in0=h1p,
                    scalar1=ff1b_sb[:, jc:jc + 1], scalar2=0.0,
                    op0=mybir.AluOpType.add, op1=mybir.AluOpType.max,
                )
                for mc in range(KC_H2):
                    nc.tensor.matmul(
                        h2_ps[:, mc, :],
                        ff2_sb[:, jc, mc * P:(mc + 1) * P],
                        h1r_sb[:, jc, :],
                        start=(jc == 0), stop=(jc == KC_H - 1),
                    )
                # interleave we-dependent work behind the early jc stages
                if jc == 1:
                    for fc in range(KC_H):
                        nc.vector.tensor_mul(
                            out=sqwe_sb[:, fc, :], in0=we_sb[:, fc, :],
                            in1=we_sb[:, fc, :],
                        )
                if jc == 2:
                    for cs in range(NCS):
                        csl = slice(cs * P, (cs + 1) * P)
                        for fc in range(KC_H):
                            nc.tensor.matmul(
                                rawT_ps[cs], we_sb[:, fc, csl], lwg_sb[:, fc, :],
                                start=False, stop=False,
                            )
                            nc.tensor.matmul(
                                sums_ps[cs][:, 0:1], we_sb[:, fc, csl], ones_col,
                                start=False, stop=False,
                            )
                            nc.tensor.matmul(
                                sums_ps[cs][:, 1:2], sqwe_sb[:, fc, csl], ones_col,
                                start=False, stop=False,
                            )

            # ---- h2 epilogue: bias, squares, rawT/sums accumulation -------
            xh2_sb = singles.tile([P, KC_H2, S], BF16)
            sqh2_sb = singles.tile([P, KC_H2, S], BF16)
            for mc in range(KC_H2):
                nc.vector.tensor_scalar(
                    out=xh2_sb[:, mc, :], in0=h2_ps[:, mc, :],
                    scalar1=ff2b_sb[:, mc:mc + 1], scalar2=None,
                    op0=mybir.AluOpType.add,
                )
                nc.scalar.square(out=sqh2_sb[:, mc, :], in_=xh2_sb[:, mc, :])
            for cs in range(NCS):
                csl = slice(cs * P, (cs + 1) * P)
                for mc in range(KC_H2):
                    nc.tensor.matmul(
                        rawT_ps[cs], xh2_sb[:, mc, csl], lwg_sb[:, KC_H + mc, :],
                        start=False, stop=False,
                    )
                    nc.tensor.matmul(
                        sums_ps[cs][:, 0:1], xh2_sb[:, mc, csl], ones_col,
                        start=False, stop=False,
                    )
                    nc.tensor.matmul(
                        sums_ps[cs][:, 1:2], sqh2_sb[:, mc, csl], ones_col,
                        start=False,
                        stop=(cs == NCS - 1 and mc == KC_H2 - 1),
                    )

            # ---- stats (positions on partitions) --------------------------
            mu_f = singles.tile([P, NCS], F32)
            mu_bf = singles.tile([P, NCS], BF16)
            ex2 = singles.tile([P, NCS], F32)
            var = singles.tile([P, NCS], F32)
            rstd = singles.tile([P, NCS], F32)
            for cs in range(NCS):
                nc.vector.tensor_scalar_mul(
                    out=mu_f[:, cs:cs + 1], in0=sums_ps[cs][:, 0:1],
                    scalar1=1.0 / NEW_H,
                )
                nc.vector.tensor_scalar_mul(
                    out=ex2[:, cs:cs + 1], in0=sums_ps[cs][:, 1:2],
                    scalar1=1.0 / NEW_H,
                )
            mu2 = singles.tile([P, NCS], F32)
            nc.vector.tensor_mul(out=mu2, in0=mu_f, in1=mu_f)
            nc.vector.tensor_sub(out=var, in0=ex2, in1=mu2)
            sd = singles.tile([P, NCS], F32)
            nc.scalar.activation(
                out=sd, in_=var, func=SQRT, bias=eps_col, scale=1.0,
            )
            nc.vector.reciprocal(out=rstd, in_=sd)

            # ---- final: fT = (rawT + mu*c1) * rstd + c2, DMA out ----------
            fT_sb = singles.tile([P, NCS, NL], F32)
            muc1 = singles.tile([P, NCS, NL], F32)
            for cs in range(NCS):
                nc.vector.tensor_scalar_mul(
                    out=muc1[:, cs, :], in0=c1b_sb,
                    scalar1=mu_f[:, cs:cs + 1],
                )
                nc.vector.tensor_add(
                    out=fT_sb[:, cs, :], in0=rawT_ps[cs], in1=muc1[:, cs, :],
                )
                nc.vector.tensor_scalar_mul(
                    out=fT_sb[:, cs, :], in0=fT_sb[:, cs, :],
                    scalar1=rstd[:, cs:cs + 1],
                )
                nc.vector.tensor_add(
                    out=fT_sb[:, cs, :], in0=fT_sb[:, cs, :], in1=c2b_sb,
                )
            nc.sync.dma_start(out=out[:, 0:2, :], in_=fT_sb[:, 0:2, :])
            nc.scalar.dma_start(out=out[:, 2:4, :], in_=fT_sb[:, 2:4, :])

    nc.compile()
    return nc


def _chunked(a, kc):
    """[kc*128, N...] -> [128, kc, N...] (partition-major chunk layout)."""
    return np.ascontiguousarray(
        a.reshape(kc, P, *a.shape[1:]).transpose(1, 0, *range(2, a.ndim + 1))
    )


_CACHE = {}


def kernel(**inputs) -> np.ndarray:
    bfl = ml_dtypes.bfloat16
    fp8 = ml_dtypes.float8_e4m3fn
    we = np.asarray(inputs["word_embedding"], np.float32)
    te = np.asarray(inputs["tag_embedding"], np.float32)
    ipw = np.asarray(inputs["in_proj_w"], np.float32)
    ipb = np.asarray(inputs["in_proj_b"], np.float32)
    opw = np.asarray(inputs["out_proj_w"], np.float32)
    ob_ = np.asarray(inputs["out_proj_b"], np.float32)
    f1w = np.asarray(inputs["ff1_w"], np.float32)
    f1b = np.asarray(inputs["ff1_b"], np.float32)
    f2w = np.asarray(inputs["ff2_w"], np.float32)
    f2b = np.asarray(inputs["ff2_b"], np.float32)
    lg = np.asarray(inputs["ln_g"], np.float32)
    lb = np.asarray(inputs["ln_b"], np.float32)
    lw = np.asarray(inputs["lin_w"], np.float32)
    lbias = np.asarray(inputs["lin_b"], np.float32)
    sb = np.asarray(inputs["span_batch"]).astype(np.int64)
    st = np.asarray(inputs["span_tag"]).astype(np.int64)
    ss = np.asarray(inputs["span_start"]).astype(np.int64)
    se = np.asarray(inputs["span_end"]).astype(np.int64)

    counts_per_b = np.bincount(sb, minlength=B)
    n_span_tiles = max(1, int(np.ceil(counts_per_b.max() / P)))
    n_pad = n_span_tiles * P

    wv_t = _chunked(ipw[2 * H:].T.astype(bfl), KC_H)
    bv_col = np.ascontiguousarray(ipb[2 * H:].reshape(KC_H, P).T)
    ops_t = _chunked((opw.T / FF1_SCALE).astype(bfl), KC_H)
    obs_col = np.ascontiguousarray((ob_ / FF1_SCALE).reshape(KC_H, P).T)
    tagT = _chunked(te.T.astype(bfl), KC_H)
    # ff1q[p, jc, t*6+hc, j] = ff1.T[t*H + hc*128 + p, jc*128 + j] * 256
    ff1T = (f1w.T * FF1_SCALE).astype(fp8)          # [T*H, H]
    ff1q = np.ascontiguousarray(
        ff1T.reshape(T * KC_H, P, KC_H, P).transpose(1, 2, 0, 3)
    )
    ff1b_col = np.ascontiguousarray(f1b.reshape(KC_H, P).T)
    ff2t = _chunked(f2w.T.astype(bfl), KC_H)
    ff2b_col = np.ascontiguousarray(f2b.reshape(KC_H2, P).T)
    lwg_full = (lw.T * lg[:, None]).astype(bfl)     # [NEW_H, NL]
    lwg_c = _chunked(lwg_full, KC_F)
    c1 = -(lwg_full.astype(np.float32).sum(0))
    c1b = np.ascontiguousarray(np.broadcast_to(c1, (P, NL)).astype(np.float32))
    c2 = lw @ lb + lbias                            # [NL]
    c2b = np.ascontiguousarray(np.broadcast_to(c2, (P, NL)).astype(np.float32))
    ident = np.eye(P, dtype=bfl)
    iota_s = np.ascontiguousarray(
        np.broadcast_to(np.arange(S, dtype=np.float16), (P, S))
    )
    iota_t = np.ascontiguousarray(
        np.broadcast_to(np.arange(T, dtype=np.float16), (P, T))
    )

    in_maps = []
    for c in range(NCORES):
        idx = np.where(sb == c)[0]
        n = len(idx)
        sps = np.zeros(n_pad, np.float32)
        spe = np.zeros(n_pad, np.float32)
        spt = np.zeros(n_pad, np.float32)
        sps[:n] = ss[idx]
        spe[:n] = se[idx]
        spt[:n] = st[idx]
        in_maps.append(dict(
            wv_t=wv_t, bv_col=bv_col, ops_t=ops_t, obs_col=obs_col,
            tagT=tagT, ff1q=ff1q, ff1b_col=ff1b_col,
            ff2t=ff2t, ff2b_col=ff2b_col,
            we_t=_chunked(np.ascontiguousarray(we[c].T).astype(bfl), KC_H),
            lwg=lwg_c, c1b=c1b, c2b=c2b, ident=ident,
            sp_start=np.ascontiguousarray(sps.reshape(n_span_tiles, P).T),
            sp_end=np.ascontiguousarray(spe.reshape(n_span_tiles, P).T),
            sp_tag=np.ascontiguousarray(spt.reshape(n_span_tiles, P).T),
            iota_s=iota_s, iota_t=iota_t,
        ))

    if n_span_tiles not in _CACHE:
        _CACHE[n_span_tiles] = build_kernel(n_span_tiles)
    nc = _CACHE[n_span_tiles]

    res = run_bass_kernel_spmd(nc, in_maps, list(range(NCORES)))
    out = np.stack([
        res.results[c]["out"].transpose(1, 0, 2).reshape(S, NL)
        for c in range(NCORES)
    ])
    return out.astype(np.float32)


if __name__ == "__main__":
    import reference
    inp = {k: np.asarray(v) for k, v in reference.setup_inputs().items()}
    got = kernel(**inp)
    print("kernel output:", got.shape, got.dtype)


# revision 10
# speedup vs baseline: 1.9468x; 1.1656x over previous
"""Trainium2 Bass kernel for nn_Estor_concat (scatter_memory).

Fully-local formulation (no collective, no cross-core traffic):
  v_tag  = tag_emb @ Wc.T + bc      with Wc = (out_proj_w @ Wv) / 256
           folded on the host (one [T,H] stage instead of two).
  W_eff[t, j] = sum_h v_tag[t, h] * ff1qT[t*H+h, j]
           where ff1qT = ff1_w.T * 256 quantized to fp8-e4m3; every core
           computes the FULL W_eff from the fp8 matrix (9.4 MB/core)
           instead of AllGather-ing tag shards (the collective's fixed
           ~15 us launch cost dominates any sharded variant).
  counts[t, s] = #spans covering s = PE-accumulated (onehot x (iota<end))
           minus (onehot x (iota<start)) over 128-span tiles.
  h1 = relu(W_eff.T @ counts + b1); h2 = ff2 @ h1 + b2
  LayerNorm + output projection evaluated TRANSPOSED (positions on
  partitions) so the stats chain is partition-parallel:
    rawT[s, l] = sum_f x[f, s]*lwg[f, l]          (lwg = lin_w.T * ln_g)
    out[s, l]  = (rawT[s, l] + mu[s]*c1[l]) * rsqrt(var[s]+eps) + c2[l]

Sharding: pure data-parallel over batch (core c owns batch c); weights
replicated. DMA is spread over the three parallel queues (SP /
Activation / Pool); the fp8 ff1 is sliced per j-chunk and 3-way split
so the W_eff -> transpose -> h1 -> h2 pipeline consumes slices as they
land. Small tensors are packed into three Pool loads to avoid per-DMA
queue overhead.
"""

import ml_dtypes
import numpy as np

import concourse.bacc as bacc
import concourse.bass as bass
import concourse.mybir as mybir
import concourse.tile as tile
from concourse.bass_utils import run_bass_kernel_spmd

T, B, S, H = 16, 8, 512, 768
H2 = 384
NEW_H = H + H2          # 1152
NL = 33                 # num labels
EPS = 1e-12
NCORES = 8
KC_H = H // 128         # 6
KC_H2 = H2 // 128       # 3
KC_F = NEW_H // 128     # 9
NCS = S // 128          # 4 position chunks
P = 128
FF1_SCALE = 256.0
G = T * KC_H            # 96 ff1 row-chunks per j-chunk
GP = 24                 # Pool's share of each jc slice (SP/Act get 36 each)
GA = (G - GP) // 2      # 36

F32 = mybir.dt.float32
BF16 = mybir.dt.bfloat16
F16 = mybir.dt.float16
FP8 = mybir.dt.float8e4

SQRT = mybir.ActivationFunctionType.Sqrt

# pk32 layout (f32 columns)
PK_BC = 0               # bc (6)
PK_F1B = 6              # ff1b (6)
PK_F2B = 12             # ff2b (3)
PK_C1 = 15              # c1 broadcast (33)
PK_C2 = 48              # c2 broadcast (33)
PK_SP = 81              # spans start/end/tag (3 * nst)
# pk16 layout (bf16 columns)
PKB_TAG = 0             # tagT (6*16 = 96)
PKB_ID = 96             # identity (128)
PKB_LWG = 224           # lwg (9*33 = 297)
PKB_W = 224 + 297


def build_kernel(n_span_tiles: int):
    nst = n_span_tiles
    nc = bacc.Bacc(
        "TRN2",
        target_bir_lowering=False,
        debug=False,
        enable_asserts=True,
        num_devices=NCORES,
    )

    def inp(name, shape, dtype=F32):
        return nc.dram_tensor(name, list(shape), dtype, kind="ExternalInput").ap()

    wc_t = inp("wc_t", (P, KC_H, H), BF16)       # (opw @ Wv).T / 256 chunked
    ff1q = inp("ff1q", (P, KC_H, G, P), FP8)     # ff1.T*256 [h, jc, t*6+hc, j]
    ff2t = inp("ff2t", (P, KC_H, H2), BF16)      # ff2.T chunked
    we_t = inp("we_t", (P, KC_H, S), BF16)       # word_embedding[b].T chunked
    pk32 = inp("pk32", (P, PK_SP + 3 * nst))
    pk16 = inp("pk16", (P, PKB_W), BF16)
    pkh16 = inp("pkh16", (P, S + T), F16)        # iota_s | iota_t

    out = nc.dram_tensor("out", [P, NCS, NL], F32, kind="ExternalOutput").ap()

    with tile.TileContext(nc) as tc:
        with (
            tc.tile_pool(name="singles", bufs=1) as singles,
            tc.tile_pool(name="spans", bufs=3) as spans,
            tc.tile_pool(name="ps_h2", bufs=1, space="PSUM") as ps_h2,
            tc.tile_pool(name="ps_big", bufs=1, space="PSUM") as ps_big,
            tc.tile_pool(name="ps_acc", bufs=1, space="PSUM") as ps_acc,
            tc.tile_pool(name="ps_sm", bufs=1, space="PSUM") as ps_sm,
        ):
            # ---- tiny constants -------------------------------------------
            ones_col = singles.tile([P, 1], BF16)
            nc.vector.memset(ones_col, 1.0)
            eps_col = singles.tile([P, 1], F32)
            nc.vector.memset(eps_col, EPS)
            scratch = singles.tile([1, 1], F32)
            zrow = singles.tile([1, NCS * (NL + 2)], BF16)
            nc.vector.memset(zrow, 0.0)

            # ---- SBUF destinations ----------------------------------------
            pk32_sb = singles.tile([P, PK_SP + 3 * nst], F32)
            pk16_sb = singles.tile([P, PKB_W], BF16)
            pkh_sb = singles.tile([P, S + T], F16)
            wc_sb = singles.tile([P, KC_H, H], BF16)
            we_sb = singles.tile([P, KC_H, S], BF16)
            ff2_sb = singles.tile([P, KC_H, H2], BF16)
            ff1_sb = singles.tile([P, KC_H, G, P], FP8)

            bc_col = pk32_sb[:, PK_BC:PK_BC + KC_H]
            ff1b_col = pk32_sb[:, PK_F1B:PK_F1B + KC_H]
            ff2b_col = pk32_sb[:, PK_F2B:PK_F2B + KC_H2]
            c1b_sb = pk32_sb[:, PK_C1:PK_C1 + NL]
            c2b_sb = pk32_sb[:, PK_C2:PK_C2 + NL]
            sps_sb = pk32_sb[:, PK_SP:PK_SP + nst]
            spe_sb = pk32_sb[:, PK_SP + nst:PK_SP + 2 * nst]
            spt_sb = pk32_sb[:, PK_SP + 2 * nst:PK_SP + 3 * nst]
            ident_sb = pk16_sb[:, PKB_ID:PKB_ID + P]
            iota_s_sb = pkh_sb[:, 0:S]
            iota_t_sb = pkh_sb[:, S:S + T]

            def tag_hc(hc):
                return pk16_sb[:, PKB_TAG + hc * T:PKB_TAG + (hc + 1) * T]

            def lwg_fc(fc):
                return pk16_sb[:, PKB_LWG + fc * NL:PKB_LWG + (fc + 1) * NL]

            # ---- DMA schedule (3 parallel queues) -------------------------
            # Pool: packs, then its share of each ff1 jc-slice, then ff2/we
            nc.gpsimd.dma_start(out=pkh_sb, in_=pkh16)
            nc.gpsimd.dma_start(out=pk32_sb, in_=pk32)
            nc.gpsimd.dma_start(out=pk16_sb, in_=pk16)
            # SP / Act: half of Wc each, then their ff1 shares
            nc.sync.dma_start(out=wc_sb[:, 0:3, :], in_=wc_t[:, 0:3, :])
            nc.scalar.dma_start(out=wc_sb[:, 3:6, :], in_=wc_t[:, 3:6, :])
            for jc in range(KC_H):
                nc.gpsimd.dma_start(
                    out=ff1_sb[:, jc, 0:GP, :], in_=ff1q[:, jc, 0:GP, :]
                )
                nc.sync.dma_start(
                    out=ff1_sb[:, jc, GP:GP + GA, :],
                    in_=ff1q[:, jc, GP:GP + GA, :],
                )
                nc.scalar.dma_start(
                    out=ff1_sb[:, jc, GP + GA:G, :],
                    in_=ff1q[:, jc, GP + GA:G, :],
                )
            nc.gpsimd.dma_start(out=ff2_sb, in_=ff2t)
            nc.gpsimd.dma_start(out=we_sb, in_=we_t)
            # act table prefetch (Sqrt/Square share a set); queues behind the
            # Act DMAs, well before the stats need it
            nc.scalar.activation(out=scratch, in_=eps_col[0:1, :], func=SQRT)

            # ---- counts (DVE compares + PE accumulation) ------------------
            counts_ps = ps_acc.tile([T, S], F32, tag="counts")
            for i in range(nst):
                lt_e = spans.tile([P, S], BF16, tag="lt_e")
                lt_s = spans.tile([P, S], BF16, tag="lt_s")
                nc.vector.tensor_scalar(
                    out=lt_e, in0=iota_s_sb, scalar1=spe_sb[:, i:i + 1],
                    scalar2=None, op0=mybir.AluOpType.is_lt,
                )
                nc.vector.tensor_scalar(
                    out=lt_s, in0=iota_s_sb, scalar1=sps_sb[:, i:i + 1],
                    scalar2=None, op0=mybir.AluOpType.is_lt,
                )
                oh_p = spans.tile([P, T], BF16, tag="oh_p")
                oh_n = spans.tile([P, T], BF16, tag="oh_n")
                nc.vector.tensor_scalar(
                    out=oh_p, in0=iota_t_sb, scalar1=spt_sb[:, i:i + 1],
                    scalar2=None, op0=mybir.AluOpType.is_equal,
                )
                nc.vector.tensor_scalar(
                    out=oh_n, in0=iota_t_sb, scalar1=spt_sb[:, i:i + 1],
                    scalar2=-1.0, op0=mybir.AluOpType.is_equal,
                    op1=mybir.AluOpType.mult,
                )
                nc.tensor.matmul(
                    counts_ps, oh_p, lt_e, start=(i == 0), stop=False,
                )
                nc.tensor.matmul(
                    counts_ps, oh_n, lt_s, start=False, stop=(i == nst - 1),
                )
            counts_sb = singles.tile([T, S], BF16)
            nc.vector.tensor_copy(out=counts_sb, in_=counts_ps)

            # ---- v_tag chain (single stage thanks to host-folded Wc) ------
            vtT_sb = singles.tile([P, KC_H, T], BF16)
            for jc in range(KC_H):
                ps = ps_sm.tile([P, T], F32, tag="sm", name=f"psvt{jc}")
                for hc in range(KC_H):
                    nc.tensor.matmul(
                        ps, wc_sb[:, hc, jc * P:(jc + 1) * P], tag_hc(hc),
                        start=(hc == 0), stop=(hc == KC_H - 1),
                    )
                nc.vector.tensor_scalar(
                    out=vtT_sb[:, jc, :], in0=ps,
                    scalar1=bc_col[:, jc:jc + 1], scalar2=None,
                    op0=mybir.AluOpType.add,
                )

            # ---- persistent accumulators ----------------------------------
            h2_ps = ps_h2.tile([P, KC_H2, S], F32)          # 3 banks
            # one bank: [cs, 0:NL] = rawT, [cs, NL:NL+2] = (sum, sumsq).
            # The whole bank is ONE accumulation group (psum zero regions
            # are bank-granular): a zeroing matmul opens it, every
            # rawT/sums matmul joins with start=False, the last one stops.
            acc_ps = ps_acc.tile([P, NCS, NL + 2], F32)
            rawT_ps = [acc_ps[:, cs, 0:NL] for cs in range(NCS)]
            sums_ps = [acc_ps[:, cs, NL:NL + 2] for cs in range(NCS)]
            nc.tensor.matmul(
                acc_ps[:, :, :], zrow[:, 0:P], zrow, start=True, stop=False,
            )

            sqwe_sb = singles.tile([P, KC_H, S], BF16)
            h1r_sb = singles.tile([P, KC_H, S], BF16)

            # ---- per-jc pipeline ------------------------------------------
            # PE: weff(jc) -> transpose(jc) -> h1(jc) -> h2(jc-1); the h2
            # accumulation trails one stage so relu(jc) never blocks the
            # next slice's W_eff work.
            def h2_accum(jc):
                for mc in range(KC_H2):
                    nc.tensor.matmul(
                        h2_ps[:, mc, :],
                        ff2_sb[:, jc, mc * P:(mc + 1) * P],
                        h1r_sb[:, jc, :],
                        start=(jc == 0), stop=(jc == KC_H - 1),
                    )

            for jc in range(KC_H):
                wps = ps_sm.tile([P, T], F32, tag="sm", name=f"wps{jc}")
                for t in range(T):
                    for hc in range(KC_H):
                        nc.tensor.matmul(
                            wps[:, t:t + 1],
                            ff1_sb[:, jc, t * KC_H + hc, :],
                            vtT_sb[:, hc, t:t + 1],
                            start=(hc == 0), stop=(hc == KC_H - 1),
                        )
                wbf = spans.tile([P, T], BF16, tag="wbf")
                nc.vector.tensor_copy(out=wbf, in_=wps)
                tp = ps_sm.tile([T, P], BF16, tag="tp", name=f"tp{jc}")
                nc.tensor.transpose(tp, wbf, ident_sb)
                wrow = spans.tile([T, P], BF16, tag="wrow")
                nc.vector.tensor_copy(out=wrow, in_=tp)
                h1p = ps_big.tile([P, S], F32, tag="big", name=f"h1p{jc}")
                nc.tensor.matmul(h1p, wrow, counts_sb, start=True, stop=True)
                if jc > 0:
                    h2_accum(jc - 1)
                nc.vector.tensor_scalar(
                    out=h1r_sb[:, jc, :], in0=h1p,
                    scalar1=ff1b_col[:, jc:jc + 1], scalar2=0.0,
                    op0=mybir.AluOpType.add, op1=mybir.AluOpType.max,
                )
                if jc == 1:
                    # word-embedding squares while the queues stream ff1
                    for fc in range(KC_H):
                        nc.vector.tensor_mul(
                            out=sqwe_sb[:, fc, :], in0=we_sb[:, fc, :],
                            in1=we_sb[:, fc, :],
                        )
            h2_accum(KC_H - 1)

            # ---- h2 epilogue ----------------------------------------------
            xh2_sb = singles.tile([P, KC_H2, S], BF16)
            sqh2_sb = singles.tile([P, KC_H2, S], BF16)
            for mc in range(KC_H2):
                nc.vector.tensor_scalar(
                    out=xh2_sb[:, mc, :], in0=h2_ps[:, mc, :],
                    scalar1=ff2b_col[:, mc:mc + 1], scalar2=None,
                    op0=mybir.AluOpType.add,
                )
                nc.vector.tensor_mul(
                    out=sqh2_sb[:, mc, :], in0=xh2_sb[:, mc, :],
                    in1=xh2_sb[:, mc, :],
                )

            # ---- rawT / sums accumulation (transposed output path) --------
            for cs in range(NCS):
                csl = slice(cs * P, (cs + 1) * P)
                for fc in range(KC_H):
                    nc.tensor.matmul(
                        rawT_ps[cs], we_sb[:, fc, csl], lwg_fc(fc),
                        start=False, stop=False,
                    )
                    nc.tensor.matmul(
                        sums_ps[cs][:, 0:1], we_sb[:, fc, csl], ones_col,
                        start=False, stop=False,
                    )
                    nc.tensor.matmul(
                        sums_ps[cs][:, 1:2], sqwe_sb[:, fc, csl], ones_col,
                        start=False, stop=False,
                    )
                for mc in range(KC_H2):
                    nc.tensor.matmul(
                        rawT_ps[cs], xh2_sb[:, mc, csl], lwg_fc(KC_H + mc),
                        start=False, stop=False,
                    )
                    nc.tensor.matmul(
                        sums_ps[cs][:, 0:1], xh2_sb[:, mc, csl], ones_col,
                        start=False, stop=False,
                    )
                    nc.tensor.matmul(
                        sums_ps[cs][:, 1:2], sqh2_sb[:, mc, csl], ones_col,
                        start=False,
                        stop=(cs == NCS - 1 and mc == KC_H2 - 1),
                    )

            # ---- stats (positions on partitions) --------------------------
            mu_f = singles.tile([P, NCS], F32)
            ex2 = singles.tile([P, NCS], F32)
            var = singles.tile([P, NCS], F32)
            rstd = singles.tile([P, NCS], F32)
            for cs in range(NCS):
                nc.vector.tensor_scalar_mul(
                    out=mu_f[:, cs:cs + 1], in0=sums_ps[cs][:, 0:1],
                    scalar1=1.0 / NEW_H,
                )
                nc.vector.tensor_scalar_mul(
                    out=ex2[:, cs:cs + 1], in0=sums_ps[cs][:, 1:2],
                    scalar1=1.0 / NEW_H,
                )
            mu2 = singles.tile([P, NCS], F32)
            nc.vector.tensor_mul(out=mu2, in0=mu_f, in1=mu_f)
            nc.vector.tensor_sub(out=var, in0=ex2, in1=mu2)
            sd = singles.tile([P, NCS], F32)
            nc.scalar.activation(
                out=sd, in_=var, func=SQRT, bias=eps_col, scale=1.0,
            )
            nc.vector.reciprocal(out=rstd, in_=sd)

            # ---- final: fT = (rawT + mu*c1) * rstd + c2, DMA out ----------
            fT_sb = singles.tile([P, NCS, NL], F32)
            muc1 = singles.tile([P, NCS, NL], F32)
            for cs in range(NCS):
                nc.vector.tensor_scalar_mul(
                    out=muc1[:, cs, :], in0=c1b_sb,
                    scalar1=mu_f[:, cs:cs + 1],
                )
                nc.vector.tensor_add(
                    out=fT_sb[:, cs, :], in0=rawT_ps[cs], in1=muc1[:, cs, :],
                )
                nc.vector.tensor_scalar_mul(
                    out=fT_sb[:, cs, :], in0=fT_sb[:, cs, :],
                    scalar1=rstd[:, cs:cs + 1],
                )
                nc.vector.tensor_add(
                    out=fT_sb[:, cs, :], in0=fT_sb[:, cs, :], in1=c2b_sb,
                )
                if cs == 1:
                    nc.sync.dma_start(out=out[:, 0:2, :], in_=fT_sb[:, 0:2, :])
            nc.scalar.dma_start(out=out[:, 2:4, :], in_=fT_sb[:, 2:4, :])

    nc.compile()
    return nc


def _chunked(a, kc):
    """[kc*128, N...] -> [128, kc, N...] (partition-major chunk layout)."""
    return np.ascontiguousarray(
        a.reshape(kc, P, *a.shape[1:]).transpose(1, 0, *range(2, a.ndim + 1))
    )


_CACHE = {}


def kernel(**inputs) -> np.ndarray:
    bfl = ml_dtypes.bfloat16
    fp8 = ml_dtypes.float8_e4m3fn
    we = np.asarray(inputs["word_embedding"], np.float32)
    te = np.asarray(inputs["tag_embedding"], np.float32)
    ipw = np.asarray(inputs["in_proj_w"], np.float32)
    ipb = np.asarray(inputs["in_proj_b"], np.float32)
    opw = np.asarray(inputs["out_proj_w"], np.float32)
    ob_ = np.asarray(inputs["out_proj_b"], np.float32)
    f1w = np.asarray(inputs["ff1_w"], np.float32)
    f1b = np.asarray(inputs["ff1_b"], np.float32)
    f2w = np.asarray(inputs["ff2_w"], np.float32)
    f2b = np.asarray(inputs["ff2_b"], np.float32)
    lg = np.asarray(inputs["ln_g"], np.float32)
    lb = np.asarray(inputs["ln_b"], np.float32)
    lw = np.asarray(inputs["lin_w"], np.float32)
    lbias = np.asarray(inputs["lin_b"], np.float32)
    sb = np.asarray(inputs["span_batch"]).astype(np.int64)
    st = np.asarray(inputs["span_tag"]).astype(np.int64)
    ss = np.asarray(inputs["span_start"]).astype(np.int64)
    se = np.asarray(inputs["span_end"]).astype(np.int64)

    counts_per_b = np.bincount(sb, minlength=B)
    n_span_tiles = max(1, int(np.ceil(counts_per_b.max() / P)))
    n_pad = n_span_tiles * P

    Wv = ipw[2 * H:]
    bv = ipb[2 * H:]
    wc = (opw @ Wv) / FF1_SCALE                    # [H, H]
    bc = (bv @ opw.T + ob_) / FF1_SCALE            # [H]
    wc_t = _chunked(wc.T.astype(bfl), KC_H)
    ff1T = (f1w.T * FF1_SCALE).astype(fp8)         # [T*H, H]
    ff1q = np.ascontiguousarray(
        ff1T.reshape(G, P, KC_H, P).transpose(1, 2, 0, 3)
    )
    ff2t = _chunked(f2w.T.astype(bfl), KC_H)
    lwg_full = (lw.T * lg[:, None]).astype(bfl)    # [NEW_H, NL]
    c1 = -(lwg_full.astype(np.float32).sum(0))
    c2 = lw @ lb + lbias

    pk32_w = PK_SP + 3 * n_span_tiles
    pk32_common = np.zeros((P, PK_SP), np.float32)
    pk32_common[:, PK_BC:PK_BC + KC_H] = bc.reshape(KC_H, P).T
    pk32_common[:, PK_F1B:PK_F1B + KC_H] = f1b.reshape(KC_H, P).T
    pk32_common[:, PK_F2B:PK_F2B + KC_H2] = f2b.reshape(KC_H2, P).T
    pk32_common[:, PK_C1:PK_C1 + NL] = c1
    pk32_common[:, PK_C2:PK_C2 + NL] = c2

    pk16 = np.zeros((P, PKB_W), bfl)
    # tagT: [p, hc*16+t] = te.T[hc*128+p, t]
    pk16[:, PKB_TAG:PKB_TAG + G] = (
        te.T.astype(bfl).reshape(KC_H, P, T).transpose(1, 0, 2).reshape(P, G)
    )
    pk16[:, PKB_ID:PKB_ID + P] = np.eye(P, dtype=bfl)
    pk16[:, PKB_LWG:PKB_LWG + KC_F * NL] = (
        lwg_full.reshape(KC_F, P, NL).transpose(1, 0, 2).reshape(P, KC_F * NL)
    )

    pkh16 = np.zeros((P, S + T), np.float16)
    pkh16[:, 0:S] = np.arange(S, dtype=np.float16)
    pkh16[:, S:S + T] = np.arange(T, dtype=np.float16)

    in_maps = []
    for c in range(NCORES):
        idx = np.where(sb == c)[0]
        n = len(idx)
        sps = np.zeros(n_pad, np.float32)
        spe = np.zeros(n_pad, np.float32)
        spt = np.zeros(n_pad, np.float32)
        sps[:n] = ss[idx]
        spe[:n] = se[idx]
        spt[:n] = st[idx]
        pk32c = np.zeros((P, pk32_w), np.float32)
        pk32c[:, :PK_SP] = pk32_common
        pk32c[:, PK_SP:PK_SP + n_span_tiles] = sps.reshape(n_span_tiles, P).T
        pk32c[:, PK_SP + n_span_tiles:PK_SP + 2 * n_span_tiles] = (
            spe.reshape(n_span_tiles, P).T
        )
        pk32c[:, PK_SP + 2 * n_span_tiles:] = spt.reshape(n_span_tiles, P).T
        in_maps.append(dict(
            wc_t=wc_t, ff1q=ff1q, ff2t=ff2t,
            we_t=_chunked(np.ascontiguousarray(we[c].T).astype(bfl), KC_H),
            pk32=pk32c, pk16=pk16, pkh16=pkh16,
        ))

    if n_span_tiles not in _CACHE:
        _CACHE[n_span_tiles] = build_kernel(n_span_tiles)
    nc = _CACHE[n_span_tiles]

    res = run_bass_kernel_spmd(nc, in_maps, list(range(NCORES)))
    out = np.stack([
        res.results[c]["out"].transpose(1, 0, 2).reshape(S, NL)
        for c in range(NCORES)
    ])
    return out.astype(np.float32)


if __name__ == "__main__":
    import reference
    inp = {k: np.asarray(v) for k, v in reference.setup_inputs().items()}
    got = kernel(**inp)
    print("kernel output:", got.shape, got.dtype)


# revision 12
# speedup vs baseline: 2.0716x; 1.0641x over previous
"""Trainium2 Bass kernel for nn_Estor_concat (scatter_memory).

Fully-local formulation (no collective, no cross-core traffic):
  v_tag  = tag_emb @ Wc.T + bc      with Wc = (out_proj_w @ Wv) / 256
           folded on the host (one [T,H] stage instead of two).
  W_eff[t, j] = sum_h v_tag[t, h] * ff1qT[t*H+h, j]
           where ff1qT = ff1_w.T * 256 quantized to fp8-e4m3; every core
           computes the FULL W_eff from the fp8 matrix (9.4 MB/core)
           instead of AllGather-ing tag shards (the collective's fixed
           ~15 us launch cost dominates any sharded variant).
  counts[t, s] = #spans covering s = PE-accumulated (onehot x (iota<end))
           minus (onehot x (iota<start)) over 128-span tiles.
  h1 = relu(W_eff.T @ counts + b1); h2 = ff2 @ h1 + b2
  LayerNorm + output projection evaluated TRANSPOSED (positions on
  partitions) so the stats chain is partition-parallel:
    rawT[s, l] = sum_f x[f, s]*lwg[f, l]          (lwg = lin_w.T * ln_g)
    out[s, l]  = (rawT[s, l] + mu[s]*c1[l]) * rsqrt(var[s]+eps) + c2[l]

Sharding: pure data-parallel over batch (core c owns batch c); weights
replicated. DMA is spread over the three parallel queues (SP /
Activation / Pool); the fp8 ff1 is sliced per j-chunk and 3-way split
so the W_eff -> transpose -> h1 -> h2 pipeline consumes slices as they
land. Small tensors are packed into three Pool loads to avoid per-DMA
queue overhead.
"""

import ml_dtypes
import numpy as np

import concourse.bacc as bacc
import concourse.bass as bass
import concourse.mybir as mybir
import concourse.tile as tile
from concourse.bass_utils import run_bass_kernel_spmd

T, B, S, H = 16, 8, 512, 768
H2 = 384
NEW_H = H + H2          # 1152
NL = 33                 # num labels
EPS = 1e-12
NCORES = 8
KC_H = H // 128         # 6
KC_H2 = H2 // 128       # 3
KC_F = NEW_H // 128     # 9
NCS = S // 128          # 4 position chunks
P = 128
FF1_SCALE = 256.0
G = T * KC_H            # 96 ff1 row-chunks per j-chunk
GS = 27                 # SP share of each jc slice
GA = 33                 # Act share
GP = G - GS - GA        # Pool share (36)

F32 = mybir.dt.float32
BF16 = mybir.dt.bfloat16
F16 = mybir.dt.float16
FP8 = mybir.dt.float8e4

SQRT = mybir.ActivationFunctionType.Sqrt

# pk32 layout (f32 columns)
PK_BC = 0               # bc (6)
PK_F1B = 6              # ff1b (6)
PK_F2B = 12             # ff2b (3)
PK_C1 = 15              # c1 broadcast (33)
PK_C2 = 48              # c2 broadcast (33)
PK_SP = 81              # spans start/end/tag (3 * nst)
# pk16 layout (bf16 columns)
PKB_TAG = 0             # tagT (6*16 = 96)
PKB_ID = 96             # identity (128)
PKB_LWG = 224           # lwg (9*33 = 297)
PKB_W = 224 + 297


def build_kernel(n_span_tiles: int):
    nst = n_span_tiles
    nc = bacc.Bacc(
        "TRN2",
        target_bir_lowering=False,
        debug=False,
        enable_asserts=True,
        num_devices=NCORES,
    )

    def inp(name, shape, dtype=F32):
        return nc.dram_tensor(name, list(shape), dtype, kind="ExternalInput").ap()

    wc_t = inp("wc_t", (P, KC_H, H), BF16)       # (opw @ Wv).T / 256 chunked
    ff1q = inp("ff1q", (P, KC_H, G, P), FP8)     # ff1.T*256 [h, jc, t*6+hc, j]
    ff2t = inp("ff2t", (P, KC_H, H2), BF16)      # ff2.T chunked
    we_t = inp("we_t", (P, KC_H, S), BF16)       # word_embedding[b].T chunked
    pk32 = inp("pk32", (P, PK_SP + 3 * nst))
    pk16 = inp("pk16", (P, PKB_W), BF16)
    pkh16 = inp("pkh16", (P, S + T), F16)        # iota_s | iota_t

    out = nc.dram_tensor("out", [P, NCS, NL], F32, kind="ExternalOutput").ap()

    with tile.TileContext(nc) as tc:
        with (
            tc.tile_pool(name="singles", bufs=1) as singles,
            tc.tile_pool(name="spans", bufs=3) as spans,
            tc.tile_pool(name="ps_h2", bufs=1, space="PSUM") as ps_h2,
            tc.tile_pool(name="ps_big", bufs=1, space="PSUM") as ps_big,
            tc.tile_pool(name="ps_acc", bufs=1, space="PSUM") as ps_acc,
            tc.tile_pool(name="ps_sm", bufs=1, space="PSUM") as ps_sm,
        ):
            # ---- tiny constants -------------------------------------------
            ones_col = singles.tile([P, 1], BF16)
            nc.vector.memset(ones_col, 1.0)
            eps_col = singles.tile([P, 1], F32)
            nc.vector.memset(eps_col, EPS)
            scratch = singles.tile([1, 1], F32)
            zrow = singles.tile([1, NCS * (NL + 2)], BF16)
            nc.vector.memset(zrow, 0.0)

            # ---- SBUF destinations ----------------------------------------
            pk32_sb = singles.tile([P, PK_SP + 3 * nst], F32)
            pk16_sb = singles.tile([P, PKB_W], BF16)
            pkh_sb = singles.tile([P, S + T], F16)
            wc_sb = singles.tile([P, KC_H, H], BF16)
            we_sb = singles.tile([P, KC_H, S], BF16)
            ff2_sb = singles.tile([P, KC_H, H2], BF16)
            ff1_sb = singles.tile([P, KC_H, G, P], FP8)

            bc_col = pk32_sb[:, PK_BC:PK_BC + KC_H]
            ff1b_col = pk32_sb[:, PK_F1B:PK_F1B + KC_H]
            ff2b_col = pk32_sb[:, PK_F2B:PK_F2B + KC_H2]
            c1b_sb = pk32_sb[:, PK_C1:PK_C1 + NL]
            c2b_sb = pk32_sb[:, PK_C2:PK_C2 + NL]
            sps_sb = pk32_sb[:, PK_SP:PK_SP + nst]
            spe_sb = pk32_sb[:, PK_SP + nst:PK_SP + 2 * nst]
            spt_sb = pk32_sb[:, PK_SP + 2 * nst:PK_SP + 3 * nst]
            ident_sb = pk16_sb[:, PKB_ID:PKB_ID + P]
            iota_s_sb = pkh_sb[:, 0:S]
            iota_t_sb = pkh_sb[:, S:S + T]

            def tag_hc(hc):
                return pk16_sb[:, PKB_TAG + hc * T:PKB_TAG + (hc + 1) * T]

            def lwg_fc(fc):
                return pk16_sb[:, PKB_LWG + fc * NL:PKB_LWG + (fc + 1) * NL]

            # ---- DMA schedule (3 parallel queues, balanced finish) --------
            # Pool: packs then ff1 shares; SP: wc/2, jc0 share, ff2, rest;
            # Act: wc/2 then shares; we halves trail on SP/Act (needed late).
            nc.gpsimd.dma_start(out=pkh_sb, in_=pkh16)
            nc.gpsimd.dma_start(out=pk32_sb, in_=pk32)
            nc.gpsimd.dma_start(out=pk16_sb, in_=pk16)
            nc.sync.dma_start(out=wc_sb[:, 0:3, :], in_=wc_t[:, 0:3, :])
            nc.scalar.dma_start(out=wc_sb[:, 3:6, :], in_=wc_t[:, 3:6, :])
            for jc in range(KC_H):
                nc.sync.dma_start(
                    out=ff1_sb[:, jc, 0:GS, :], in_=ff1q[:, jc, 0:GS, :]
                )
                nc.scalar.dma_start(
                    out=ff1_sb[:, jc, GS:GS + GA, :],
                    in_=ff1q[:, jc, GS:GS + GA, :],
                )
                nc.gpsimd.dma_start(
                    out=ff1_sb[:, jc, GS + GA:G, :],
                    in_=ff1q[:, jc, GS + GA:G, :],
                )
                if jc == 0:
                    nc.sync.dma_start(out=ff2_sb, in_=ff2t)
            nc.sync.dma_start(out=we_sb[:, 0:3, :], in_=we_t[:, 0:3, :])
            nc.scalar.dma_start(out=we_sb[:, 3:6, :], in_=we_t[:, 3:6, :])
            # act table prefetch (Sqrt/Square/Relu share a set); queues behind
            # the Act DMAs, well before the Act relus/squares need it
            nc.scalar.activation(out=scratch, in_=eps_col[0:1, :], func=SQRT)

            # ---- counts (DVE compares + PE accumulation) ------------------
            counts_ps = ps_acc.tile([T, S], F32, tag="counts")
            for i in range(nst):
                lt_e = spans.tile([P, S], BF16, tag="lt_e")
                lt_s = spans.tile([P, S], BF16, tag="lt_s")
                nc.vector.tensor_scalar(
                    out=lt_e, in0=iota_s_sb, scalar1=spe_sb[:, i:i + 1],
                    scalar2=None, op0=mybir.AluOpType.is_lt,
                )
                nc.vector.tensor_scalar(
                    out=lt_s, in0=iota_s_sb, scalar1=sps_sb[:, i:i + 1],
                    scalar2=None, op0=mybir.AluOpType.is_lt,
                )
                oh_p = spans.tile([P, T], BF16, tag="oh_p")
                oh_n = spans.tile([P, T], BF16, tag="oh_n")
                nc.vector.tensor_scalar(
                    out=oh_p, in0=iota_t_sb, scalar1=spt_sb[:, i:i + 1],
                    scalar2=None, op0=mybir.AluOpType.is_equal,
                )
                nc.vector.tensor_scalar(
                    out=oh_n, in0=iota_t_sb, scalar1=spt_sb[:, i:i + 1],
                    scalar2=-1.0, op0=mybir.AluOpType.is_equal,
                    op1=mybir.AluOpType.mult,
                )
                nc.tensor.matmul(
                    counts_ps, oh_p, lt_e, start=(i == 0), stop=False,
                )
                nc.tensor.matmul(
                    counts_ps, oh_n, lt_s, start=False, stop=(i == nst - 1),
                )
            counts_sb = singles.tile([T, S], BF16)
            nc.vector.tensor_copy(out=counts_sb, in_=counts_ps)

            # ---- v_tag chain (single stage thanks to host-folded Wc) ------
            vtT_sb = singles.tile([P, KC_H, T], BF16)
            for jc in range(KC_H):
                ps = ps_sm.tile([P, T], F32, tag="sm", name=f"psvt{jc}")
                for hc in range(KC_H):
                    nc.tensor.matmul(
                        ps, wc_sb[:, hc, jc * P:(jc + 1) * P], tag_hc(hc),
                        start=(hc == 0), stop=(hc == KC_H - 1),
                    )
                nc.vector.tensor_scalar(
                    out=vtT_sb[:, jc, :], in0=ps,
                    scalar1=bc_col[:, jc:jc + 1], scalar2=None,
                    op0=mybir.AluOpType.add,
                )

            # ---- persistent accumulators ----------------------------------
            h2_ps = ps_h2.tile([P, KC_H2, S], F32)          # 3 banks
            # one bank: [cs, 0:NL] = rawT, [cs, NL:NL+2] = (sum, sumsq).
            # The whole bank is ONE accumulation group (psum zero regions
            # are bank-granular): a zeroing matmul opens it, every
            # rawT/sums matmul joins with start=False, the last one stops.
            acc_ps = ps_acc.tile([P, NCS, NL + 2], F32)
            rawT_ps = [acc_ps[:, cs, 0:NL] for cs in range(NCS)]
            sums_ps = [acc_ps[:, cs, NL:NL + 2] for cs in range(NCS)]
            nc.tensor.matmul(
                acc_ps[:, :, :], zrow[:, 0:P], zrow, start=True, stop=False,
            )

            sqwe_sb = singles.tile([P, KC_H, S], BF16)
            h1r_sb = singles.tile([P, KC_H, S], BF16)

            # ---- per-jc pipeline ------------------------------------------
            # PE: weff(jc) -> transpose(jc) -> h1(jc) -> h2(jc-1); the h2
            # accumulation trails one stage so relu(jc) never blocks the
            # next slice's W_eff work.
            def h2_accum(jc):
                for mc in range(KC_H2):
                    nc.tensor.matmul(
                        h2_ps[:, mc, :],
                        ff2_sb[:, jc, mc * P:(mc + 1) * P],
                        h1r_sb[:, jc, :],
                        start=(jc == 0), stop=(jc == KC_H - 1),
                    )

            for jc in range(KC_H):
                wps = ps_sm.tile([P, T], F32, tag="sm", name=f"wps{jc}")
                for t in range(T):
                    for hc in range(KC_H):
                        nc.tensor.matmul(
                            wps[:, t:t + 1],
                            ff1_sb[:, jc, t * KC_H + hc, :],
                            vtT_sb[:, hc, t:t + 1],
                            start=(hc == 0), stop=(hc == KC_H - 1),
                        )
                wbf = spans.tile([P, T], BF16, tag="wbf")
                nc.vector.tensor_copy(out=wbf, in_=wps)
                tp = ps_sm.tile([T, P], BF16, tag="tp", name=f"tp{jc}")
                nc.tensor.transpose(tp, wbf, ident_sb)
                wrow = spans.tile([T, P], BF16, tag="wrow")
                nc.vector.tensor_copy(out=wrow, in_=tp)
                h1p = ps_big.tile([P, S], F32, tag="big", name=f"h1p{jc}")
                nc.tensor.matmul(h1p, wrow, counts_sb, start=True, stop=True)
                if jc > 0:
                    h2_accum(jc - 1)
                if jc >= 4:
                    nc.scalar.activation(
                        out=h1r_sb[:, jc, :], in_=h1p,
                        func=mybir.ActivationFunctionType.Relu,
                        bias=ff1b_col[:, jc:jc + 1], scale=1.0,
                    )
                else:
                    nc.vector.tensor_scalar(
                        out=h1r_sb[:, jc, :], in0=h1p,
                        scalar1=ff1b_col[:, jc:jc + 1], scalar2=0.0,
                        op0=mybir.AluOpType.add, op1=mybir.AluOpType.max,
                    )
                if jc == 1:
                    # word-embedding squares while the queues stream ff1
                    for fc in range(KC_H):
                        nc.vector.tensor_mul(
                            out=sqwe_sb[:, fc, :], in0=we_sb[:, fc, :],
                            in1=we_sb[:, fc, :],
                        )
            h2_accum(KC_H - 1)

            # ---- h2 epilogue ----------------------------------------------
            xh2_sb = singles.tile([P, KC_H2, S], BF16)
            sqh2_sb = singles.tile([P, KC_H2, S], BF16)
            bias_eng = [nc.vector, nc.vector, nc.vector]
            sq_eng = [nc.scalar, nc.scalar, nc.vector]
            for mc in range(KC_H2):
                bias_eng[mc].tensor_scalar(
                    out=xh2_sb[:, mc, :], in0=h2_ps[:, mc, :],
                    scalar1=ff2b_col[:, mc:mc + 1], scalar2=None,
                    op0=mybir.AluOpType.add,
                )
                if sq_eng[mc] is nc.scalar:
                    nc.scalar.square(
                        out=sqh2_sb[:, mc, :], in_=xh2_sb[:, mc, :]
                    )
                else:
                    sq_eng[mc].tensor_mul(
                        out=sqh2_sb[:, mc, :], in0=xh2_sb[:, mc, :],
                        in1=xh2_sb[:, mc, :],
                    )

            # ---- rawT / sums accumulation (transposed output path) --------
            for cs in range(NCS):
                csl = slice(cs * P, (cs + 1) * P)
                for fc in range(KC_H):
                    nc.tensor.matmul(
                        rawT_ps[cs], we_sb[:, fc, csl], lwg_fc(fc),
                        start=False, stop=False,
                    )
                    nc.tensor.matmul(
                        sums_ps[cs][:, 0:1], we_sb[:, fc, csl], ones_col,
                        start=False, stop=False,
                    )
                    nc.tensor.matmul(
                        sums_ps[cs][:, 1:2], sqwe_sb[:, fc, csl], ones_col,
                        start=False, stop=False,
                    )
                for mc in range(KC_H2):
                    nc.tensor.matmul(
                        rawT_ps[cs], xh2_sb[:, mc, csl], lwg_fc(KC_H + mc),
                        start=False, stop=False,
                    )
                    nc.tensor.matmul(
                        sums_ps[cs][:, 0:1], xh2_sb[:, mc, csl], ones_col,
                        start=False, stop=False,
                    )
                    nc.tensor.matmul(
                        sums_ps[cs][:, 1:2], sqh2_sb[:, mc, csl], ones_col,
                        start=False,
                        stop=(cs == NCS - 1 and mc == KC_H2 - 1),
                    )

            # ---- stats (positions on partitions) --------------------------
            mu_f = singles.tile([P, NCS], F32)
            ex2 = singles.tile([P, NCS], F32)
            var = singles.tile([P, NCS], F32)
            rstd = singles.tile([P, NCS], F32)
            for cs in range(NCS):
                nc.vector.tensor_scalar_mul(
                    out=mu_f[:, cs:cs + 1], in0=sums_ps[cs][:, 0:1],
                    scalar1=1.0 / NEW_H,
                )
                nc.vector.tensor_scalar_mul(
                    out=ex2[:, cs:cs + 1], in0=sums_ps[cs][:, 1:2],
                    scalar1=1.0 / NEW_H,
                )
            mu2 = singles.tile([P, NCS], F32)
            nc.vector.tensor_mul(out=mu2, in0=mu_f, in1=mu_f)
            nc.vector.tensor_sub(out=var, in0=ex2, in1=mu2)
            sd = singles.tile([P, NCS], F32)
            nc.scalar.activation(
                out=sd, in_=var, func=SQRT, bias=eps_col, scale=1.0,
            )
            nc.vector.reciprocal(out=rstd, in_=sd)

            # ---- final: fT = (rawT + mu*c1) * rstd + c2, DMA out ----------
            fT_sb = singles.tile([P, NCS, NL], F32)
            muc1 = singles.tile([P, NCS, NL], F32)
            for cs in range(NCS):
                nc.vector.tensor_scalar_mul(
                    out=muc1[:, cs, :], in0=c1b_sb,
                    scalar1=mu_f[:, cs:cs + 1],
                )
                nc.vector.tensor_add(
                    out=fT_sb[:, cs, :], in0=rawT_ps[cs], in1=muc1[:, cs, :],
                )
                nc.vector.tensor_scalar_mul(
                    out=fT_sb[:, cs, :], in0=fT_sb[:, cs, :],
                    scalar1=rstd[:, cs:cs + 1],
                )
                nc.vector.tensor_add(
                    out=fT_sb[:, cs, :], in0=fT_sb[:, cs, :], in1=c2b_sb,
                )
                if cs == 1:
                    nc.sync.dma_start(out=out[:, 0:2, :], in_=fT_sb[:, 0:2, :])
            nc.scalar.dma_start(out=out[:, 2:4, :], in_=fT_sb[:, 2:4, :])

    nc.compile()
    return nc


def _chunked(a, kc):
    """[kc*128, N...] -> [128, kc, N...] (partition-major chunk layout)."""
    return np.ascontiguousarray(
        a.reshape(kc, P, *a.shape[1:]).transpose(1, 0, *range(2, a.ndim + 1))
    )


_CACHE = {}


def kernel(**inputs) -> np.ndarray:
    bfl = ml_dtypes.bfloat16
    fp8 = ml_dtypes.float8_e4m3fn
    we = np.asarray(inputs["word_embedding"], np.float32)
    te = np.asarray(inputs["tag_embedding"], np.float32)
    ipw = np.asarray(inputs["in_proj_w"], np.float32)
    ipb = np.asarray(inputs["in_proj_b"], np.float32)
    opw = np.asarray(inputs["out_proj_w"], np.float32)
    ob_ = np.asarray(inputs["out_proj_b"], np.float32)
    f1w = np.asarray(inputs["ff1_w"], np.float32)
    f1b = np.asarray(inputs["ff1_b"], np.float32)
    f2w = np.asarray(inputs["ff2_w"], np.float32)
    f2b = np.asarray(inputs["ff2_b"], np.float32)
    lg = np.asarray(inputs["ln_g"], np.float32)
    lb = np.asarray(inputs["ln_b"], np.float32)
    lw = np.asarray(inputs["lin_w"], np.float32)
    lbias = np.asarray(inputs["lin_b"], np.float32)
    sb = np.asarray(inputs["span_batch"]).astype(np.int64)
    st = np.asarray(inputs["span_tag"]).astype(np.int64)
    ss = np.asarray(inputs["span_start"]).astype(np.int64)
    se = np.asarray(inputs["span_end"]).astype(np.int64)

    counts_per_b = np.bincount(sb, minlength=B)
    n_span_tiles = max(1, int(np.ceil(counts_per_b.max() / P)))
    n_pad = n_span_tiles * P

    Wv = ipw[2 * H:]
    bv = ipb[2 * H:]
    wc = (opw @ Wv) / FF1_SCALE                    # [H, H]
    bc = (bv @ opw.T + ob_) / FF1_SCALE            # [H]
    wc_t = _chunked(wc.T.astype(bfl), KC_H)
    ff1T = (f1w.T * FF1_SCALE).astype(fp8)         # [T*H, H]
    ff1q = np.ascontiguousarray(
        ff1T.reshape(G, P, KC_H, P).transpose(1, 2, 0, 3)
    )
    ff2t = _chunked(f2w.T.astype(bfl), KC_H)
    lwg_full = (lw.T * lg[:, None]).astype(bfl)    # [NEW_H, NL]
    c1 = -(lwg_full.astype(np.float32).sum(0))
    c2 = lw @ lb + lbias

    pk32_w = PK_SP + 3 * n_span_tiles
    pk32_common = np.zeros((P, PK_SP), np.float32)
    pk32_common[:, PK_BC:PK_BC + KC_H] = bc.reshape(KC_H, P).T
    pk32_common[:, PK_F1B:PK_F1B + KC_H] = f1b.reshape(KC_H, P).T
    pk32_common[:, PK_F2B:PK_F2B + KC_H2] = f2b.reshape(KC_H2, P).T
    pk32_common[:, PK_C1:PK_C1 + NL] = c1
    pk32_common[:, PK_C2:PK_C2 + NL] = c2

    pk16 = np.zeros((P, PKB_W), bfl)
    # tagT: [p, hc*16+t] = te.T[hc*128+p, t]
    pk16[:, PKB_TAG:PKB_TAG + G] = (
        te.T.astype(bfl).reshape(KC_H, P, T).transpose(1, 0, 2).reshape(P, G)
    )
    pk16[:, PKB_ID:PKB_ID + P] = np.eye(P, dtype=bfl)
    pk16[:, PKB_LWG:PKB_LWG + KC_F * NL] = (
        lwg_full.reshape(KC_F, P, NL).transpose(1, 0, 2).reshape(P, KC_F * NL)
    )

    pkh16 = np.zeros((P, S + T), np.float16)
    pkh16[:, 0:S] = np.arange(S, dtype=np.float16)
    pkh16[:, S:S + T] = np.arange(T, dtype=np.float16)

    in_maps = []
    for c in range(NCORES):
        idx = np.where(sb == c)[0]
        n = len(idx)
        sps = np.zeros(n_pad, np.float32)
        spe = np.zeros(n_pad, np.float32)
        spt = np.zeros(n_pad, np.float32)
        sps[:n] = ss[idx]
        spe[:n] = se[idx]
        spt[:n] = st[idx]
        pk32c = np.zeros((P, pk32_w), np.float32)
        pk32c[:, :PK_SP] = pk32_common
        pk32c[:, PK_SP:PK_SP + n_span_tiles] = sps.reshape(n_span_tiles, P).T
        pk32c[:, PK_SP + n_span_tiles:PK_SP + 2 * n_span_tiles] = (
            spe.reshape(n_span_tiles, P).T
        )
        pk32c[:, PK_SP + 2 * n_span_tiles:] = spt.reshape(n_span_tiles, P).T
        in_maps.append(dict(
            wc_t=wc_t, ff1q=ff1q, ff2t=ff2t,
            we_t=_chunked(np.ascontiguousarray(we[c].T).astype(bfl), KC_H),
            pk32=pk32c, pk16=pk16, pkh16=pkh16,
        ))

    if n_span_tiles not in _CACHE:
        _CACHE[n_span_tiles] = build_kernel(n_span_tiles)
    nc = _CACHE[n_span_tiles]

    res = run_bass_kernel_spmd(nc, in_maps, list(range(NCORES)))
    out = np.stack([
        res.results[c]["out"].transpose(1, 0, 2).reshape(S, NL)
        for c in range(NCORES)
    ])
    return out.astype(np.float32)


if __name__ == "__main__":
    import reference
    inp = {k: np.asarray(v) for k, v in reference.setup_inputs().items()}
    got = kernel(**inp)
    print("kernel output:", got.shape, got.dtype)
